# revision 17
# baseline (speedup 1.0000x reference)
"""Trainium2 Bass kernel for nn_DeformRouting (deformable routing conv).

Sharding: 8 cores, data-parallel over N x H-halves: core c handles image
n = c//2, row-half = c%2 (14 rows x 28 cols = 392 pixels).

Math (per pixel pt, output channel o; m = c*9+kk):
  out[o, pt] = x[o,pt] * sum_m w_wgt[o,m] * samp[m,pt]
             + sum_m b_wgt.reshape(64,576)[o,m] * samp[m,pt]
where samp[(c,kk), pt] is the bilinear sample of x[c] at the deformed
position of tap kk for pixel pt.

Device pipeline per core (points-on-partitions, 4 chunks of 98 pts):
  1. offset conv: 4 PE matmuls -> ps_off[pt, ch, 18] (PSUM, read in place).
  2. index math (short chain, all coords pre-shifted positive so
     floor == i32-trunc): ix = off*13.5 + base' -> floor -> clip ->
     idx = 29*ycl + xcl (fp16, exact ints).
  3. idx wrap to the gather's 16-partition layout: 8 PE perm-matmuls
     (fp16) + one tensor_scalar(-928) PSUM->i16 copy.
  4. per-chunk SWDGE dma_gather (1152 idx x 1KB) from a host-built
     29x29 2x2-PATCH table (fp16): one descriptor fetches all four
     bilinear taps [A0|A1|B0|B1] x 64ch. Chunk gathers pipeline: chunk
     c's DMA + combine run under chunk c+1's descriptor generation.
  5. bilinear weights (fp16, off critical path) + combine (7 DVE ops
     per chunk, fp16).
  6. per-chunk PE transposes (fp16) -> rhs[(c,kk), pt] and 10
     accumulating fp16 matmuls -> ps1/ps2; out = ps1*x + ps2 (f32).
"""

import numpy as np

import concourse.bass as bass
import concourse.tile as tile
from concourse import bacc, mybir
from concourse.bass_utils import run_bass_kernel_spmd

# problem constants (hardcoded per contract)
N, CIN, COUT, H, W, K = 4, 64, 64, 28, 28, 3
K2 = K * K  # 9
NCORES = 8
HHALF = H // 2          # 14 rows per core
NPT = HHALF * W         # 392 points per core
PCH = 98                # points per partition-chunk
NCH = 4                 # chunks (4*98 = 392)
TBL_S = H + 1           # 29 y-floor slots
TBL_T = W + 1           # 29 x-floor slots
TBL_ROWS = TBL_S * TBL_T  # 841 patch rows
SC = (W - 1) / 2.0      # 13.5
XOFF = 30.0             # x floor-shift: xf = round(ix + 30 - 0.5) = floor(ix) + 30
YOFF = 32.0             # y floor-shift: yf = round(iy + 32 - 0.5) = floor(iy) + 32
# The DVE f32->i32 cast rounds to nearest, so the host bakes (XOFF - 0.5)
# into the base grid and the fractional weight adds the 0.5 back.
# idx = 29*(ycl-31) + (xcl-29) = 29*ycl + xcl - 928
IDX_BIAS = 928.0
NI = K2 * 128           # 1152 gather descriptors per chunk
NB = 5                  # ceil(576/128) contraction chunks

F32 = mybir.dt.float32
F16 = mybir.dt.float16
I32 = mybir.dt.int32
I16 = mybir.dt.int16

_CACHE = {}


def _alu(name):
    return getattr(mybir.AluOpType, name)


def _build_program():
    nc = bacc.Bacc("TRN2", target_bir_lowering=False, debug=False,
                   num_devices=NCORES)

    # DRAM I/O (per-core shapes)
    tbl = nc.dram_tensor("tbl", [TBL_ROWS, 4 * CIN], F16, kind="ExternalInput")
    xc16 = nc.dram_tensor("xc16", [128, 512], F16, kind="ExternalInput")
    xcf = nc.dram_tensor("xcf", [COUT, NPT], F32, kind="ExternalInput")
    wofft = nc.dram_tensor("wofft", [128, 2 * K2], F16, kind="ExternalInput")
    basex = nc.dram_tensor("basex", [128, NCH * K2], F32, kind="ExternalInput")
    basey = nc.dram_tensor("basey", [128, NCH * K2], F32, kind="ExternalInput")
    wwb = nc.dram_tensor("wwb", [128, 10 * COUT], F16, kind="ExternalInput")
    mg = nc.dram_tensor("mg", [128, 8 * 128], F16, kind="ExternalInput")
    out_d = nc.dram_tensor("out", [COUT, NPT], F32, kind="ExternalOutput")

    mult, add, sub = _alu("mult"), _alu("add"), _alu("subtract")
    is_eq = _alu("is_equal")
    amin, amax = _alu("min"), _alu("max")

    with tile.TileContext(nc) as tc:
        with (
            tc.tile_pool(name="const", bufs=1) as cpool,
            tc.tile_pool(name="work", bufs=1) as wpool,
            tc.tile_pool(name="psoff", bufs=1, space="PSUM") as opool,
            tc.tile_pool(name="pso", bufs=1, space="PSUM") as popool,
        ):
            # ---- constant / input loads (conv deps first) ----
            xc_sb = cpool.tile([128, 512], F16)
            nc.sync.dma_start(xc_sb[:], xc16.ap())
            wofft_sb = cpool.tile([128, 2 * K2], F16)
            nc.sync.dma_start(wofft_sb[:], wofft.ap())
            basex_sb = cpool.tile([128, NCH, K2], F32)
            nc.sync.dma_start(basex_sb[:], basex.ap().rearrange(
                "p (a b) -> p a b", a=NCH))
            basey_sb = cpool.tile([128, NCH, K2], F32)
            nc.sync.dma_start(basey_sb[:], basey.ap().rearrange(
                "p (a b) -> p a b", a=NCH))
            mg_sb = cpool.tile([128, 8, 128], F16)
            nc.sync.dma_start(mg_sb[:], mg.ap().rearrange(
                "p (a b) -> p a b", a=8))
            wwb_sb = cpool.tile([128, 10, COUT], F16)
            nc.sync.dma_start(wwb_sb[:], wwb.ap().rearrange(
                "p (a b) -> p a b", a=10))
            xcf_sb = cpool.tile([COUT, NPT], F32)
            nc.sync.dma_start(xcf_sb[:], xcf.ap())

            # ---- 1. offset conv: ps_off[pt, ch, 18] ----
            ps_off = opool.tile([128, NCH, 2 * K2], F32)
            for ch in range(NCH):
                nc.tensor.matmul(
                    out=ps_off[:, ch, :],
                    lhsT=xc_sb[:, ch * PCH:ch * PCH + 128],
                    rhs=wofft_sb[:],
                    start=True, stop=True,
                )

            # ---- 2. index math (critical path to the gathers) ----
            shp = [128, NCH, K2]

            def t32(name):
                return wpool.tile(shp, F32, name=name)

            offx = ps_off[:, :, 0:2 * K2:2]
            offy = ps_off[:, :, 1:2 * K2:2]
            ix = t32("ix")
            nc.vector.scalar_tensor_tensor(ix[:], offx, SC, basex_sb[:],
                                           mult, add)
            iy = t32("iy")
            nc.vector.scalar_tensor_tensor(iy[:], offy, SC, basey_sb[:],
                                           mult, add)
            xi = wpool.tile(shp, I32, name="xi")
            nc.vector.tensor_copy(xi[:], ix[:])
            xf = t32("xf")
            nc.vector.tensor_copy(xf[:], xi[:])
            yi = wpool.tile(shp, I32, name="yi")
            nc.vector.tensor_copy(yi[:], iy[:])
            yf = t32("yf")
            nc.vector.tensor_copy(yf[:], yi[:])
            xcl = t32("xcl")
            nc.vector.tensor_scalar(xcl[:], xf[:], 57.0, 29.0, amin, amax)
            ycl = t32("ycl")
            nc.vector.tensor_scalar(ycl[:], yf[:], 59.0, 31.0, amin, amax)
            idxh = wpool.tile(shp, F16, name="idxh")
            nc.vector.scalar_tensor_tensor(idxh[:], ycl[:], float(TBL_T),
                                           xcl[:], mult, add)

            # ---- 3. wrap idx into the gather's 16-partition layout ----
            psw = opool.tile([128, 8, NCH * K2], F32, name="psw")
            idxv = idxh[:].rearrange("p a b -> p (a b)")
            for gsel in range(8):
                nc.tensor.matmul(
                    out=psw[:, gsel, :], lhsT=mg_sb[:, gsel, :], rhs=idxv,
                    start=True, stop=True)
            wrap = wpool.tile([128, NCH * K2, 8], I16, name="wrap")
            for ch in range(NCH):
                nc.vector.tensor_scalar(
                    wrap[:, ch * K2:(ch + 1) * K2, :].rearrange(
                        "q m g -> q g m"),
                    psw[:, :, ch * K2:(ch + 1) * K2], IDX_BIAS, None, sub)

            # ---- 4+5. per-chunk gather + combine weights ----
            # bilinear weights (fp16) - consumed only by the combine, so the
            # scheduler runs these during gather descriptor generation.
            wx1 = t32("wx1")
            nc.vector.scalar_tensor_tensor(wx1[:], ix[:], 0.5, xf[:],
                                           add, sub)
            wy1 = t32("wy1")
            nc.vector.scalar_tensor_tensor(wy1[:], iy[:], 0.5, yf[:],
                                           add, sub)
            wx0 = t32("wx0")
            nc.vector.tensor_scalar(wx0[:], wx1[:], -1.0, 1.0, mult, add)
            wy0 = t32("wy0")
            nc.vector.tensor_scalar(wy0[:], wy1[:], -1.0, 1.0, mult, add)

            def valid(f, lo, hi, name):
                c = t32(name + "c")
                nc.vector.tensor_scalar(c[:], f[:], hi, lo, amin, amax)
                v = t32(name)
                nc.vector.tensor_tensor(v[:], c[:], f[:], is_eq)
                return v

            vx0 = valid(xf, XOFF, 27.0 + XOFF, "vx0")
            vx1 = valid(xf, XOFF - 1.0, 26.0 + XOFF, "vx1")
            vy0 = valid(yf, YOFF, 27.0 + YOFF, "vy0")
            vy1 = valid(yf, YOFF - 1.0, 26.0 + YOFF, "vy1")

            def vmul(a, b, name, fp16=False):
                o = wpool.tile(shp, F16 if fp16 else F32, name=name)
                nc.vector.tensor_tensor(o[:], a[:], b[:], mult)
                return o

            wx0v = vmul(wx0, vx0, "wx0v")
            wx1v = vmul(wx1, vx1, "wx1v")
            wy0v = vmul(wy0, vy0, "wy0v")
            wy1v = vmul(wy1, vy1, "wy1v")
            w00 = vmul(wy0v, wx0v, "w00", fp16=True)
            w10 = vmul(wy1v, wx0v, "w10", fp16=True)
            w01 = vmul(wy0v, wx1v, "w01", fp16=True)
            w11 = vmul(wy1v, wx1v, "w11", fp16=True)

            # per-chunk tiles
            ps1 = popool.tile([COUT, NPT], F32, name="ps1")
            ps2 = popool.tile([COUT, NPT], F32, name="ps2")
            out_sb = wpool.tile([COUT, NPT], F32, name="out_sb")

            # kk split per chunk: half A = kk 0..3 (m-blocks 0,1), half B =
            # kk 4..8 (m-blocks 2,3,4); finer gather pipelining + short tail.
            KA = 4
            for ch in range(NCH):
                gaA = wpool.tile([128, KA, 4 * CIN], F16, name=f"gaA{ch}")
                nc.gpsimd.dma_gather(
                    out_ap=gaA[:],
                    in_ap=tbl.ap(),
                    idxs_ap=wrap[:, ch * K2:ch * K2 + KA, :].rearrange(
                        "q m g -> q (m g)"),
                    num_idxs=KA * 128, num_idxs_reg=KA * 128,
                    elem_size=4 * CIN, single_packet=False)
                gaB = wpool.tile([128, K2 - KA, 4 * CIN], F16, name=f"gaB{ch}")
                nc.gpsimd.dma_gather(
                    out_ap=gaB[:],
                    in_ap=tbl.ap(),
                    idxs_ap=wrap[:, ch * K2 + KA:(ch + 1) * K2, :].rearrange(
                        "q m g -> q (m g)"),
                    num_idxs=(K2 - KA) * 128, num_idxs_reg=(K2 - KA) * 128,
                    elem_size=4 * CIN, single_packet=False)

                # combine into samp [128, 640] fp16 viewed [p, 10, 64];
                # cols 576:640 are zero pad (m rows 576.. of the contraction)
                samp = wpool.tile([128, 640], F16, name=f"samp{ch}")
                sview = samp[:].rearrange("p (k c) -> p k c", k=10)
                nc.vector.memset(sview[:, K2, :], 0.0)
                tmp = wpool.tile([128, NB, CIN], F16, name=f"tmp{ch}")

                for half, ga, k0, nk in ((0, gaA, 0, KA),
                                         (1, gaB, KA, K2 - KA)):
                    sv = sview[:, k0:k0 + nk, :]
                    tv = tmp[:, :nk, :]

                    def bc(wt):
                        return wt[:, ch, k0:k0 + nk, None].to_broadcast(
                            [128, nk, CIN])

                    nc.vector.tensor_tensor(sv, ga[:, :, 0:CIN], bc(w00),
                                            mult)
                    nc.vector.tensor_tensor(tv, ga[:, :, CIN:2 * CIN],
                                            bc(w10), mult)
                    nc.vector.tensor_tensor(sv, sv, tv, add)
                    nc.vector.tensor_tensor(tv, ga[:, :, 2 * CIN:3 * CIN],
                                            bc(w01), mult)
                    nc.vector.tensor_tensor(sv, sv, tv, add)
                    nc.vector.tensor_tensor(tv, ga[:, :, 3 * CIN:], bc(w11),
                                            mult)
                    nc.vector.tensor_tensor(sv, sv, tv, add)

                # DMA-transpose halves: samp[p, m] -> rhs[m%128, m//128, p]
                rhs = wpool.tile([128, NB, 128], F16, name=f"rhs{ch}")
                nc.sync.dma_start(rhs[:, 0:2, :], samp[:, 0:256],
                                  transpose=True)
                nc.sync.dma_start(rhs[:, 2:NB, :], samp[:, 256:640],
                                  transpose=True)

                # final matmuls for this chunk's point columns
                cols = slice(ch * PCH, (ch + 1) * PCH)
                for b in range(NB):
                    nc.tensor.matmul(
                        out=ps1[:, cols], lhsT=wwb_sb[:, b, :],
                        rhs=rhs[:, b, :PCH],
                        start=(b == 0), stop=(b == NB - 1))
                for b in range(NB):
                    nc.tensor.matmul(
                        out=ps2[:, cols], lhsT=wwb_sb[:, NB + b, :],
                        rhs=rhs[:, b, :PCH],
                        start=(b == 0), stop=(b == NB - 1))
                nc.vector.tensor_tensor(out_sb[:, cols], ps1[:, cols],
                                        xcf_sb[:, cols], mult)
                nc.vector.tensor_tensor(out_sb[:, cols], out_sb[:, cols],
                                        ps2[:, cols], add)
                nc.sync.dma_start(out_d.ap()[:, cols], out_sb[:, cols])

    nc.compile()
    return nc


def _host_inputs(x, w_off, b_off, w_wgt, b_wgt):
    """Build the 8 per-core input dicts (layout/shard prep only)."""
    x = np.asarray(x, dtype=np.float32)
    w_off = np.asarray(w_off, dtype=np.float32)
    b_off = np.asarray(b_off, dtype=np.float32)
    w_wgt = np.asarray(w_wgt, dtype=np.float32)
    b_wgt = np.asarray(b_wgt, dtype=np.float32)

    xs = np.linspace(-1.0, 1.0, W).astype(np.float32)
    ys = np.linspace(-1.0, 1.0, H).astype(np.float32)
    kx = np.linspace(-(K - 1) / (W - 1), (K - 1) / (W - 1), K).astype(np.float32)
    ky = np.linspace(-(K - 1) / (H - 1), (K - 1) / (H - 1), K).astype(np.float32)

    # wwb [128, 10, 64] fp16, contraction rows m = kk*64 + c (kk-major, to
    # match the device's contiguous samp layout): chunks 0..4 =
    # W~[m, o] = w_wgt[o, c*9+kk] (zero-pad 576->640), chunks 5..9 =
    # B~[m, o] = b_wgt.reshape(64, 576)[o, c*9+kk].
    m_new = np.arange(K2 * CIN)
    m_old = (m_new % CIN) * K2 + (m_new // CIN)   # (kk,c) -> c*9+kk
    wtp = np.zeros((640, COUT), dtype=np.float32)
    wtp[:576] = w_wgt.T[m_old]
    btp = np.zeros((640, COUT), dtype=np.float32)
    btp[:576] = b_wgt.reshape(CIN, K2 * COUT).T[m_old]
    wwb = np.concatenate([wtp.reshape(5, 128, COUT),
                          btp.reshape(5, 128, COUT)], axis=0)
    wwb = wwb.transpose(1, 0, 2).reshape(128, 10 * COUT).astype(np.float16)

    # idx-wrap permutation selectors: mg[pt, g*128+q] = (pt == g*16 + q%16)
    mgm = np.zeros((128, 8, 128), dtype=np.float16)
    q = np.arange(128)
    for gsel in range(8):
        mgm[gsel * 16 + (q % 16), gsel, q] = 1.0
    mgm = mgm.reshape(128, 8 * 128)

    wofft = np.zeros((128, 2 * K2), dtype=np.float16)
    wofft[:CIN] = w_off.T.astype(np.float16)

    # patch-table row/col clip maps
    rt = np.clip(np.arange(TBL_S) - 1, 0, H - 1)
    rb = np.clip(np.arange(TBL_S), 0, H - 1)
    ct = np.clip(np.arange(TBL_T) - 1, 0, W - 1)
    cr = np.clip(np.arange(TBL_T), 0, W - 1)

    in_maps = []
    for c in range(NCORES):
        n, half = divmod(c, 2)
        r0 = HHALF * half
        xn = x[n]                             # [64, 28, 28]
        x_hwc = xn.transpose(1, 2, 0)         # [28, 28, 64]

        # 2x2 patch table [841, 256] fp16: row (s,t) =
        # [x[rt,ct] | x[rb,ct] | x[rt,cr] | x[rb,cr]]
        tbl = np.concatenate([
            x_hwc[rt][:, ct], x_hwc[rb][:, ct],
            x_hwc[rt][:, cr], x_hwc[rb][:, cr],
        ], axis=-1).astype(np.float16)        # [29, 29, 256]

        xslice = xn.reshape(CIN, H * W)[:, r0 * W:r0 * W + NPT]
        xcpad = np.zeros((128, 512), dtype=np.float16)
        xcpad[:CIN, :NPT] = xslice.astype(np.float16)

        # base grids [128, NCH, K2] with the floor-shift bakes (-0.5 turns
        # the round-to-nearest cast into a floor)
        bx = np.full((128, NCH, K2), SC + XOFF - 0.5, dtype=np.float32)
        by = np.full((128, NCH, K2), SC + YOFF - 0.5, dtype=np.float32)
        p_idx = np.arange(PCH)
        for ch in range(NCH):
            g = r0 * W + ch * PCH + p_idx          # global pixel
            row, col = g // W, g % W
            for kk in range(K2):
                kyi, kxi = divmod(kk, K)
                bx[:PCH, ch, kk] = (xs[col] + kx[kxi] + b_off[2 * kk]
                                    + 1.0) * SC + XOFF - 0.5
                by[:PCH, ch, kk] = (ys[row] + ky[kyi] + b_off[2 * kk + 1]
                                    + 1.0) * SC + YOFF - 0.5

        in_maps.append({
            "tbl": tbl.reshape(TBL_ROWS, 4 * CIN),
            "xc16": xcpad,
            "xcf": np.ascontiguousarray(xslice[:COUT]),
            "wofft": wofft,
            "basex": bx.reshape(128, NCH * K2),
            "basey": by.reshape(128, NCH * K2),
            "wwb": wwb,
            "mg": mgm,
        })
    return in_maps


def get_program():
    if "nc" not in _CACHE:
        _CACHE["nc"] = _build_program()
    return _CACHE["nc"]


def run_cores(in_maps, **kw):
    nc = get_program()
    return run_bass_kernel_spmd(nc, in_maps, core_ids=list(range(NCORES)), **kw)


def assemble(results):
    out = np.zeros((N, COUT, H, W), dtype=np.float32)
    for c in range(NCORES):
        n, half = divmod(c, 2)
        out[n, :, HHALF * half:HHALF * (half + 1), :] = \
            results[c]["out"].reshape(COUT, HHALF, W)
    return out


def kernel(x, w_off, b_off, w_wgt, b_wgt):
    in_maps = _host_inputs(x, w_off, b_off, w_wgt, b_wgt)
    res = run_cores(in_maps)
    return assemble(res.results)


# revision 21
# speedup vs baseline: 1.3245x; 1.3245x over previous
"""Trainium2 Bass kernel for nn_DeformRouting (deformable routing conv).

Sharding: 8 cores, data-parallel over N x H-halves: core c handles image
n = c//2, row-half = c%2 (14 rows x 28 cols = 392 pixels).

Math (per pixel pt, output channel o; m = c*9+kk):
  out[o, pt] = x[o,pt] * sum_m w_wgt[o,m] * samp[m,pt]
             + sum_m b_wgt.reshape(64,576)[o,m] * samp[m,pt]
where samp[(c,kk), pt] is the bilinear sample of x[c] at the deformed
position of tap kk for pixel pt.

Device pipeline per core (points-on-partitions, 4 chunks of 98 pts):
  1. offset conv: 4 PE matmuls -> ps_off[pt, ch, 18] (PSUM, read in place).
  2. index math (short chain, all coords pre-shifted positive so
     floor == i32-trunc): ix = off*13.5 + base' -> floor -> clip ->
     idx = 29*ycl + xcl (fp16, exact ints).
  3. idx wrap to the gather's 16-partition layout: 8 PE perm-matmuls
     (fp16) + one tensor_scalar(-928) PSUM->i16 copy.
  4. per-chunk SWDGE dma_gather (1152 idx x 1KB) from a host-built
     29x29 2x2-PATCH table (fp16): one descriptor fetches all four
     bilinear taps [A0|A1|B0|B1] x 64ch. Chunk gathers pipeline: chunk
     c's DMA + combine run under chunk c+1's descriptor generation.
  5. bilinear weights (fp16, off critical path) + combine (7 DVE ops
     per chunk, fp16).
  6. per-chunk PE transposes (fp16) -> rhs[(c,kk), pt] and 10
     accumulating fp16 matmuls -> ps1/ps2; out = ps1*x + ps2 (f32).
"""

import numpy as np

import concourse.bass as bass
import concourse.tile as tile
from concourse import bacc, mybir
from concourse.bass_utils import run_bass_kernel_spmd

# problem constants (hardcoded per contract)
N, CIN, COUT, H, W, K = 4, 64, 64, 28, 28, 3
K2 = K * K  # 9
NCORES = 8
HHALF = H // 2          # 14 rows per core
NPT = HHALF * W         # 392 points per core
PCH = 98                # points per partition-chunk
NCH = 4                 # chunks (4*98 = 392)
TBL_S = H + 1           # 29 y-floor slots
TBL_T = W + 1           # 29 x-floor slots
TBL_ROWS = TBL_S * TBL_T  # 841 patch rows
SC = (W - 1) / 2.0      # 13.5
XOFF = 30.0             # x floor-shift: xf = round(ix + 30 - 0.5) = floor(ix) + 30
YOFF = 32.0             # y floor-shift: yf = round(iy + 32 - 0.5) = floor(iy) + 32
# The DVE f32->i32 cast rounds to nearest, so the host bakes (XOFF - 0.5)
# into the base grid and the fractional weight adds the 0.5 back.
# idx = 29*(ycl-31) + (xcl-29) = 29*ycl + xcl - 928
IDX_BIAS = 928.0
NI = K2 * 128           # 1152 gather descriptors per chunk
NB = 5                  # ceil(576/128) contraction chunks

F32 = mybir.dt.float32
F16 = mybir.dt.float16
I32 = mybir.dt.int32
I16 = mybir.dt.int16

_CACHE = {}


def _alu(name):
    return getattr(mybir.AluOpType, name)


def _build_program():
    nc = bacc.Bacc("TRN2", target_bir_lowering=False, debug=False,
                   num_devices=NCORES)

    # DRAM I/O (per-core shapes)
    tbl = nc.dram_tensor("tbl", [TBL_ROWS, 4 * CIN], F16, kind="ExternalInput")
    xc16 = nc.dram_tensor("xc16", [128, 512], F16, kind="ExternalInput")
    xcf = nc.dram_tensor("xcf", [COUT, NPT], F32, kind="ExternalInput")
    wofft = nc.dram_tensor("wofft", [128, 2 * K2], F16, kind="ExternalInput")
    basex = nc.dram_tensor("basex", [128, NCH * K2], F32, kind="ExternalInput")
    basey = nc.dram_tensor("basey", [128, NCH * K2], F32, kind="ExternalInput")
    wwb = nc.dram_tensor("wwb", [128, 10 * COUT], F16, kind="ExternalInput")
    mg = nc.dram_tensor("mg", [128, 8 * 128], F16, kind="ExternalInput")
    out_d = nc.dram_tensor("out", [COUT, NPT], F32, kind="ExternalOutput")

    mult, add, sub = _alu("mult"), _alu("add"), _alu("subtract")
    is_eq = _alu("is_equal")
    amin, amax = _alu("min"), _alu("max")

    with tile.TileContext(nc) as tc:
        with (
            tc.tile_pool(name="const", bufs=1) as cpool,
            tc.tile_pool(name="work", bufs=1) as wpool,
            tc.tile_pool(name="psoff", bufs=1, space="PSUM") as opool,
            tc.tile_pool(name="pso", bufs=1, space="PSUM") as popool,
        ):
            # ---- constant / input loads (conv deps first) ----
            xc_sb = cpool.tile([128, 512], F16)
            nc.sync.dma_start(xc_sb[:], xc16.ap())
            wofft_sb = cpool.tile([128, 2 * K2], F16)
            nc.sync.dma_start(wofft_sb[:], wofft.ap())
            basex_sb = cpool.tile([128, NCH, K2], F32)
            nc.sync.dma_start(basex_sb[:], basex.ap().rearrange(
                "p (a b) -> p a b", a=NCH))
            basey_sb = cpool.tile([128, NCH, K2], F32)
            nc.sync.dma_start(basey_sb[:], basey.ap().rearrange(
                "p (a b) -> p a b", a=NCH))
            mg_sb = cpool.tile([128, 8, 128], F16)
            nc.sync.dma_start(mg_sb[:], mg.ap().rearrange(
                "p (a b) -> p a b", a=8))
            wwb_sb = cpool.tile([128, 10, COUT], F16)
            nc.sync.dma_start(wwb_sb[:], wwb.ap().rearrange(
                "p (a b) -> p a b", a=10))
            xcf_sb = cpool.tile([COUT, NPT], F32)
            nc.sync.dma_start(xcf_sb[:], xcf.ap())

            # ---- 1. offset conv: ps_off[pt, ch, 18] ----
            ps_off = opool.tile([128, NCH, 2 * K2], F32)
            for ch in range(NCH):
                nc.tensor.matmul(
                    out=ps_off[:, ch, :],
                    lhsT=xc_sb[:, ch * PCH:ch * PCH + 128],
                    rhs=wofft_sb[:],
                    start=True, stop=True,
                )

            # ---- 2. index math (critical path to the gathers) ----
            # The f32->i32 cast rounds to nearest; with the -0.5 host bake
            # that IS the floor. clip commutes with the rounding cast for
            # integer bounds, so clip+floor fuse into one ts-with-cast op.
            shp = [128, NCH, K2]

            def t32(name):
                return wpool.tile(shp, F32, name=name)

            offx = ps_off[:, :, 0:2 * K2:2]
            offy = ps_off[:, :, 1:2 * K2:2]
            ix = t32("ix")
            nc.vector.scalar_tensor_tensor(ix[:], offx, SC, basex_sb[:],
                                           mult, add)
            iy = t32("iy")
            nc.vector.scalar_tensor_tensor(iy[:], offy, SC, basey_sb[:],
                                           mult, add)
            xcl_i = wpool.tile(shp, I32, name="xcl_i")
            nc.vector.tensor_scalar(xcl_i[:], ix[:], 57.0, 29.0, amin, amax)
            ycl_i = wpool.tile(shp, I32, name="ycl_i")
            nc.vector.tensor_scalar(ycl_i[:], iy[:], 59.0, 31.0, amin, amax)
            idxh = wpool.tile(shp, F16, name="idxh")
            nc.vector.scalar_tensor_tensor(idxh[:], ycl_i[:], float(TBL_T),
                                           xcl_i[:], mult, add)

            # ---- 3. wrap idx into the gather's 16-partition layout ----
            psw = opool.tile([128, 8, NCH * K2], F32, name="psw")
            idxv = idxh[:].rearrange("p a b -> p (a b)")
            for gsel in range(8):
                nc.tensor.matmul(
                    out=psw[:, gsel, :], lhsT=mg_sb[:, gsel, :], rhs=idxv,
                    start=True, stop=True)
            wrap = wpool.tile([128, NCH * K2, 8], I16, name="wrap")
            for ch in range(NCH):
                nc.vector.tensor_scalar(
                    wrap[:, ch * K2:(ch + 1) * K2, :].rearrange(
                        "q m g -> q g m"),
                    psw[:, :, ch * K2:(ch + 1) * K2], IDX_BIAS, None, sub)

            # ---- 4+5. per-chunk gather + combine weights ----
            # bilinear weights (fp16) - consumed only by the combine, so the
            # scheduler runs these during gather descriptor generation.
            # Fractional weights use the CLIPPED floor: wherever the clip
            # bites, both taps of that axis have zero validity, so the
            # wrong magnitude is multiplied by zero.
            xcl_f = t32("xcl_f")
            nc.vector.tensor_copy(xcl_f[:], xcl_i[:])
            ycl_f = t32("ycl_f")
            nc.vector.tensor_copy(ycl_f[:], ycl_i[:])
            wx1 = t32("wx1")
            nc.vector.scalar_tensor_tensor(wx1[:], ix[:], 0.5, xcl_f[:],
                                           add, sub)
            wy1 = t32("wy1")
            nc.vector.scalar_tensor_tensor(wy1[:], iy[:], 0.5, ycl_f[:],
                                           add, sub)
            wx0 = t32("wx0")
            nc.vector.tensor_scalar(wx0[:], wx1[:], -1.0, 1.0, mult, add)
            wy0 = t32("wy0")
            nc.vector.tensor_scalar(wy0[:], wy1[:], -1.0, 1.0, mult, add)

            # validity straight from the continuous coords (parallel to the
            # idx chain): round(i) in [lo, hi] <=> i in [lo-0.5, hi+0.5)
            def valid(f, lo, hi, name):
                c = t32(name + "c")
                nc.vector.tensor_scalar(c[:], f[:], hi + 0.4999, lo - 0.5,
                                        amin, amax)
                v = t32(name)
                nc.vector.tensor_tensor(v[:], c[:], f[:], is_eq)
                return v

            vx0 = valid(ix, XOFF, 27.0 + XOFF, "vx0")
            vx1 = valid(ix, XOFF - 1.0, 26.0 + XOFF, "vx1")
            vy0 = valid(iy, YOFF, 27.0 + YOFF, "vy0")
            vy1 = valid(iy, YOFF - 1.0, 26.0 + YOFF, "vy1")

            def vmul(a, b, name, fp16=False):
                o = wpool.tile(shp, F16 if fp16 else F32, name=name)
                nc.vector.tensor_tensor(o[:], a[:], b[:], mult)
                return o

            wx0v = vmul(wx0, vx0, "wx0v")
            wx1v = vmul(wx1, vx1, "wx1v")
            wy0v = vmul(wy0, vy0, "wy0v")
            wy1v = vmul(wy1, vy1, "wy1v")
            w00 = vmul(wy0v, wx0v, "w00", fp16=True)
            w10 = vmul(wy1v, wx0v, "w10", fp16=True)
            w01 = vmul(wy0v, wx1v, "w01", fp16=True)
            w11 = vmul(wy1v, wx1v, "w11", fp16=True)

            # per-chunk tiles
            ps1 = popool.tile([COUT, NPT], F32, name="ps1")
            ps2 = popool.tile([COUT, NPT], F32, name="ps2")
            out_sb = wpool.tile([COUT, NPT], F32, name="out_sb")

            for ch in range(NCH):
                ga = wpool.tile([128, K2, 4 * CIN], F16, name=f"ga{ch}")
                nc.gpsimd.dma_gather(
                    out_ap=ga[:],
                    in_ap=tbl.ap(),
                    idxs_ap=wrap[:, ch * K2:(ch + 1) * K2, :].rearrange(
                        "q m g -> q (m g)"),
                    num_idxs=NI, num_idxs_reg=NI, elem_size=4 * CIN,
                    single_packet=False)

                # combine into samp [128, 640] fp16 viewed [p, 10, 64];
                # cols 576:640 are zero pad (m rows 576.. of the contraction)
                samp = wpool.tile([128, 640], F16, name=f"samp{ch}")
                sview = samp[:].rearrange("p (k c) -> p k c", k=10)
                nc.vector.memset(sview[:, K2, :], 0.0)
                tmp = wpool.tile([128, K2, CIN], F16, name=f"tmp{ch}")
                sv = sview[:, :K2, :]
                tv = tmp[:]

                def bc(wt):
                    return wt[:, ch, :, None].to_broadcast([128, K2, CIN])

                nc.vector.tensor_tensor(sv, ga[:, :, 0:CIN], bc(w00), mult)
                nc.vector.tensor_tensor(tv, ga[:, :, CIN:2 * CIN], bc(w10),
                                        mult)
                nc.vector.tensor_tensor(sv, sv, tv, add)
                nc.vector.tensor_tensor(tv, ga[:, :, 2 * CIN:3 * CIN],
                                        bc(w01), mult)
                nc.vector.tensor_tensor(sv, sv, tv, add)
                nc.vector.tensor_tensor(tv, ga[:, :, 3 * CIN:], bc(w11), mult)
                nc.vector.tensor_tensor(sv, sv, tv, add)

                # DMA-transpose (ACT-dispatched): samp[p, m] ->
                # rhs[m%128, m//128, p]
                rhs = wpool.tile([128, NB, 128], F16, name=f"rhs{ch}")
                nc.scalar.dma_start(rhs[:], samp[:], transpose=True)

                # final matmuls for this chunk's point columns
                cols = slice(ch * PCH, (ch + 1) * PCH)
                for b in range(NB):
                    nc.tensor.matmul(
                        out=ps1[:, cols], lhsT=wwb_sb[:, b, :],
                        rhs=rhs[:, b, :PCH],
                        start=(b == 0), stop=(b == NB - 1))
                for b in range(NB):
                    nc.tensor.matmul(
                        out=ps2[:, cols], lhsT=wwb_sb[:, NB + b, :],
                        rhs=rhs[:, b, :PCH],
                        start=(b == 0), stop=(b == NB - 1))
                nc.vector.tensor_tensor(out_sb[:, cols], ps1[:, cols],
                                        xcf_sb[:, cols], mult)
                nc.vector.tensor_tensor(out_sb[:, cols], out_sb[:, cols],
                                        ps2[:, cols], add)
                nc.sync.dma_start(out_d.ap()[:, cols], out_sb[:, cols])

    nc.compile()
    return nc


def _host_inputs(x, w_off, b_off, w_wgt, b_wgt):
    """Build the 8 per-core input dicts (layout/shard prep only)."""
    x = np.asarray(x, dtype=np.float32)
    w_off = np.asarray(w_off, dtype=np.float32)
    b_off = np.asarray(b_off, dtype=np.float32)
    w_wgt = np.asarray(w_wgt, dtype=np.float32)
    b_wgt = np.asarray(b_wgt, dtype=np.float32)

    xs = np.linspace(-1.0, 1.0, W).astype(np.float32)
    ys = np.linspace(-1.0, 1.0, H).astype(np.float32)
    kx = np.linspace(-(K - 1) / (W - 1), (K - 1) / (W - 1), K).astype(np.float32)
    ky = np.linspace(-(K - 1) / (H - 1), (K - 1) / (H - 1), K).astype(np.float32)

    # wwb [128, 10, 64] fp16, contraction rows m = kk*64 + c (kk-major, to
    # match the device's contiguous samp layout): chunks 0..4 =
    # W~[m, o] = w_wgt[o, c*9+kk] (zero-pad 576->640), chunks 5..9 =
    # B~[m, o] = b_wgt.reshape(64, 576)[o, c*9+kk].
    m_new = np.arange(K2 * CIN)
    m_old = (m_new % CIN) * K2 + (m_new // CIN)   # (kk,c) -> c*9+kk
    wtp = np.zeros((640, COUT), dtype=np.float32)
    wtp[:576] = w_wgt.T[m_old]
    btp = np.zeros((640, COUT), dtype=np.float32)
    btp[:576] = b_wgt.reshape(CIN, K2 * COUT).T[m_old]
    wwb = np.concatenate([wtp.reshape(5, 128, COUT),
                          btp.reshape(5, 128, COUT)], axis=0)
    wwb = wwb.transpose(1, 0, 2).reshape(128, 10 * COUT).astype(np.float16)

    # idx-wrap permutation selectors: mg[pt, g*128+q] = (pt == g*16 + q%16)
    mgm = np.zeros((128, 8, 128), dtype=np.float16)
    q = np.arange(128)
    for gsel in range(8):
        mgm[gsel * 16 + (q % 16), gsel, q] = 1.0
    mgm = mgm.reshape(128, 8 * 128)

    wofft = np.zeros((128, 2 * K2), dtype=np.float16)
    wofft[:CIN] = w_off.T.astype(np.float16)

    # patch-table row/col clip maps
    rt = np.clip(np.arange(TBL_S) - 1, 0, H - 1)
    rb = np.clip(np.arange(TBL_S), 0, H - 1)
    ct = np.clip(np.arange(TBL_T) - 1, 0, W - 1)
    cr = np.clip(np.arange(TBL_T), 0, W - 1)

    in_maps = []
    for c in range(NCORES):
        n, half = divmod(c, 2)
        r0 = HHALF * half
        xn = x[n]                             # [64, 28, 28]
        x_hwc = xn.transpose(1, 2, 0)         # [28, 28, 64]

        # 2x2 patch table [841, 256] fp16: row (s,t) =
        # [x[rt,ct] | x[rb,ct] | x[rt,cr] | x[rb,cr]]
        tbl = np.concatenate([
            x_hwc[rt][:, ct], x_hwc[rb][:, ct],
            x_hwc[rt][:, cr], x_hwc[rb][:, cr],
        ], axis=-1).astype(np.float16)        # [29, 29, 256]

        xslice = xn.reshape(CIN, H * W)[:, r0 * W:r0 * W + NPT]
        xcpad = np.zeros((128, 512), dtype=np.float16)
        xcpad[:CIN, :NPT] = xslice.astype(np.float16)

        # base grids [128, NCH, K2] with the floor-shift bakes (-0.5 turns
        # the round-to-nearest cast into a floor)
        bx = np.full((128, NCH, K2), SC + XOFF - 0.5, dtype=np.float32)
        by = np.full((128, NCH, K2), SC + YOFF - 0.5, dtype=np.float32)
        p_idx = np.arange(PCH)
        for ch in range(NCH):
            g = r0 * W + ch * PCH + p_idx          # global pixel
            row, col = g // W, g % W
            for kk in range(K2):
                kyi, kxi = divmod(kk, K)
                bx[:PCH, ch, kk] = (xs[col] + kx[kxi] + b_off[2 * kk]
                                    + 1.0) * SC + XOFF - 0.5
                by[:PCH, ch, kk] = (ys[row] + ky[kyi] + b_off[2 * kk + 1]
                                    + 1.0) * SC + YOFF - 0.5

        in_maps.append({
            "tbl": tbl.reshape(TBL_ROWS, 4 * CIN),
            "xc16": xcpad,
            "xcf": np.ascontiguousarray(xslice[:COUT]),
            "wofft": wofft,
            "basex": bx.reshape(128, NCH * K2),
            "basey": by.reshape(128, NCH * K2),
            "wwb": wwb,
            "mg": mgm,
        })
    return in_maps


def get_program():
    if "nc" not in _CACHE:
        _CACHE["nc"] = _build_program()
    return _CACHE["nc"]


def run_cores(in_maps, **kw):
    nc = get_program()
    return run_bass_kernel_spmd(nc, in_maps, core_ids=list(range(NCORES)), **kw)


def assemble(results):
    out = np.zeros((N, COUT, H, W), dtype=np.float32)
    for c in range(NCORES):
        n, half = divmod(c, 2)
        out[n, :, HHALF * half:HHALF * (half + 1), :] = \
            results[c]["out"].reshape(COUT, HHALF, W)
    return out


def kernel(x, w_off, b_off, w_wgt, b_wgt):
    in_maps = _host_inputs(x, w_off, b_off, w_wgt, b_wgt)
    res = run_cores(in_maps)
    return assemble(res.results)


# revision 28
# speedup vs baseline: 1.4246x; 1.0756x over previous
"""Trainium2 Bass kernel for nn_DeformRouting (deformable routing conv).

Sharding: 8 cores, data-parallel over N x H-halves: core c handles image
n = c//2, row-half = c%2 (14 rows x 28 cols = 392 pixels).

Math (per pixel pt, output channel o; m = c*9+kk):
  out[o, pt] = x[o,pt] * sum_m w_wgt[o,m] * samp[m,pt]
             + sum_m b_wgt.reshape(64,576)[o,m] * samp[m,pt]
where samp[(c,kk), pt] is the bilinear sample of x[c] at the deformed
position of tap kk for pixel pt.

Device pipeline per core (points-on-partitions, 4 chunks of 98 pts):
  1. offset conv: 4 PE matmuls -> ps_off[pt, ch, 18] (PSUM, read in place).
  2. index math (short chain, all coords pre-shifted positive so
     floor == i32-trunc): ix = off*13.5 + base' -> floor -> clip ->
     idx = 29*ycl + xcl (fp16, exact ints).
  3. idx wrap to the gather's 16-partition layout: 8 PE perm-matmuls
     (fp16) + one tensor_scalar(-928) PSUM->i16 copy.
  4. per-chunk SWDGE dma_gather (1152 idx x 1KB) from a host-built
     29x29 2x2-PATCH table (fp16): one descriptor fetches all four
     bilinear taps [A0|A1|B0|B1] x 64ch. Chunk gathers pipeline: chunk
     c's DMA + combine run under chunk c+1's descriptor generation.
  5. bilinear weights (fp16, off critical path) + combine (7 DVE ops
     per chunk, fp16).
  6. per-chunk PE transposes (fp16) -> rhs[(c,kk), pt] and 10
     accumulating fp16 matmuls -> ps1/ps2; out = ps1*x + ps2 (f32).
"""

import numpy as np

import concourse.bass as bass
import concourse.tile as tile
from concourse import bacc, mybir
from concourse.bass_utils import run_bass_kernel_spmd

# problem constants (hardcoded per contract)
N, CIN, COUT, H, W, K = 4, 64, 64, 28, 28, 3
K2 = K * K  # 9
NCORES = 8
HHALF = H // 2          # 14 rows per core
NPT = HHALF * W         # 392 points per core
PCH = 98                # points per partition-chunk
NCH = 4                 # chunks (4*98 = 392)
TBL_S = H + 1           # 29 y-floor slots
TBL_T = W + 1           # 29 x-floor slots
TBL_ROWS = TBL_S * TBL_T  # 841 patch rows
SC = (W - 1) / 2.0      # 13.5
XOFF = 30.0             # x floor-shift: xf = round(ix + 30 - 0.5) = floor(ix) + 30
YOFF = 32.0             # y floor-shift: yf = round(iy + 32 - 0.5) = floor(iy) + 32
# The DVE f32->i32 cast rounds to nearest, so the host bakes (XOFF - 0.5)
# into the base grid and the fractional weight adds the 0.5 back.
# idx = 29*(ycl-31) + (xcl-29) = 29*ycl + xcl - 928
IDX_BIAS = 928.0
NI = K2 * 128           # 1152 gather descriptors per chunk
NB = 5                  # ceil(576/128) contraction chunks

F32 = mybir.dt.float32
F16 = mybir.dt.float16
I32 = mybir.dt.int32
I16 = mybir.dt.int16

_CACHE = {}


def _alu(name):
    return getattr(mybir.AluOpType, name)


def _build_program():
    nc = bacc.Bacc("TRN2", target_bir_lowering=False, debug=False,
                   num_devices=NCORES)

    # DRAM I/O (per-core shapes)
    tbl = nc.dram_tensor("tbl", [TBL_ROWS, 4 * CIN], F16, kind="ExternalInput")
    xc16 = nc.dram_tensor("xc16", [128, 512], F16, kind="ExternalInput")
    xcf = nc.dram_tensor("xcf", [COUT, NPT], F32, kind="ExternalInput")
    wofft = nc.dram_tensor("wofft", [128, 2 * K2], F16, kind="ExternalInput")
    basex = nc.dram_tensor("basex", [128, NCH * K2], F32, kind="ExternalInput")
    basey = nc.dram_tensor("basey", [128, NCH * K2], F32, kind="ExternalInput")
    wwb = nc.dram_tensor("wwb", [128, 10 * COUT], F16, kind="ExternalInput")
    mg = nc.dram_tensor("mg", [128, 8 * 128], F16, kind="ExternalInput")
    ident = nc.dram_tensor("ident", [128, 128], F16, kind="ExternalInput")
    out_d = nc.dram_tensor("out", [COUT, NPT], F32, kind="ExternalOutput")

    mult, add, sub = _alu("mult"), _alu("add"), _alu("subtract")
    is_eq = _alu("is_equal")
    amin, amax = _alu("min"), _alu("max")

    with tile.TileContext(nc) as tc:
        with (
            tc.tile_pool(name="const", bufs=1) as cpool,
            tc.tile_pool(name="work", bufs=1) as wpool,
            tc.tile_pool(name="psoff", bufs=1, space="PSUM") as opool,
            tc.tile_pool(name="pst", bufs=2, space="PSUM") as ppool,
            tc.tile_pool(name="pso", bufs=1, space="PSUM") as popool,
        ):
            # ---- constant / input loads (conv deps first) ----
            xc_sb = cpool.tile([128, 512], F16)
            nc.sync.dma_start(xc_sb[:], xc16.ap())
            wofft_sb = cpool.tile([128, 2 * K2], F16)
            nc.sync.dma_start(wofft_sb[:], wofft.ap())
            basex_sb = cpool.tile([128, NCH, K2], F32)
            nc.sync.dma_start(basex_sb[:], basex.ap().rearrange(
                "p (a b) -> p a b", a=NCH))
            basey_sb = cpool.tile([128, NCH, K2], F32)
            nc.sync.dma_start(basey_sb[:], basey.ap().rearrange(
                "p (a b) -> p a b", a=NCH))
            mg_sb = cpool.tile([128, 8, 128], F16)
            nc.sync.dma_start(mg_sb[:], mg.ap().rearrange(
                "p (a b) -> p a b", a=8))
            wwb_sb = cpool.tile([128, 10, COUT], F16)
            nc.sync.dma_start(wwb_sb[:], wwb.ap().rearrange(
                "p (a b) -> p a b", a=10))
            xcf_sb = cpool.tile([COUT, NPT], F32)
            nc.sync.dma_start(xcf_sb[:], xcf.ap())
            id_sb = cpool.tile([128, 128], F16)
            nc.sync.dma_start(id_sb[:], ident.ap())

            # ---- 1. offset conv: ps_off[pt, ch, 18] ----
            ps_off = opool.tile([128, NCH, 2 * K2], F32)
            for ch in range(NCH):
                nc.tensor.matmul(
                    out=ps_off[:, ch, :],
                    lhsT=xc_sb[:, ch * PCH:ch * PCH + 128],
                    rhs=wofft_sb[:],
                    start=True, stop=True,
                )

            # ---- 2. index math (critical path to the gathers) ----
            # The f32->i32 cast rounds to nearest; with the -0.5 host bake
            # that IS the floor. clip commutes with the rounding cast for
            # integer bounds, so clip+floor fuse into one ts-with-cast op.
            shp = [128, NCH, K2]

            def t32(name):
                return wpool.tile(shp, F32, name=name)

            offx = ps_off[:, :, 0:2 * K2:2]
            offy = ps_off[:, :, 1:2 * K2:2]
            ix = t32("ix")
            nc.vector.scalar_tensor_tensor(ix[:], offx, SC, basex_sb[:],
                                           mult, add)
            iy = t32("iy")
            nc.vector.scalar_tensor_tensor(iy[:], offy, SC, basey_sb[:],
                                           mult, add)
            xcl_i = wpool.tile(shp, I32, name="xcl_i")
            nc.vector.tensor_scalar(xcl_i[:], ix[:], 57.0, 29.0, amin, amax)
            ycl_i = wpool.tile(shp, I32, name="ycl_i")
            nc.vector.tensor_scalar(ycl_i[:], iy[:], 59.0, 31.0, amin, amax)
            idxh = wpool.tile(shp, F16, name="idxh")
            nc.vector.scalar_tensor_tensor(idxh[:], ycl_i[:], float(TBL_T),
                                           xcl_i[:], mult, add)

            # ---- 3. wrap idx into the gather's 16-partition layout ----
            psw = opool.tile([128, 8, NCH * K2], F32, name="psw")
            idxv = idxh[:].rearrange("p a b -> p (a b)")
            for gsel in range(8):
                nc.tensor.matmul(
                    out=psw[:, gsel, :], lhsT=mg_sb[:, gsel, :], rhs=idxv,
                    start=True, stop=True)
            wrap = wpool.tile([128, NCH * K2, 8], I16, name="wrap")
            for ch in range(NCH):
                nc.vector.tensor_scalar(
                    wrap[:, ch * K2:(ch + 1) * K2, :].rearrange(
                        "q m g -> q g m"),
                    psw[:, :, ch * K2:(ch + 1) * K2], IDX_BIAS, None, sub)

            # ---- 4. per-chunk gathers, emitted before the weight math so
            # their DVE semaphore gate covers only the wrap ops ----
            gas = []
            for ch in range(NCH):
                ga = wpool.tile([128, K2, 4 * CIN], F16, name=f"ga{ch}")
                nc.gpsimd.dma_gather(
                    out_ap=ga[:],
                    in_ap=tbl.ap(),
                    idxs_ap=wrap[:, ch * K2:(ch + 1) * K2, :].rearrange(
                        "q m g -> q (m g)"),
                    num_idxs=NI, num_idxs_reg=NI, elem_size=4 * CIN,
                    single_packet=False)
                gas.append(ga)

            # ---- 4+5. per-chunk gather + combine weights ----
            # bilinear weights (fp16) - consumed only by the combine, so the
            # scheduler runs these during gather descriptor generation.
            # Fractional weights use the CLIPPED floor: wherever the clip
            # bites, both taps of that axis have zero validity, so the
            # wrong magnitude is multiplied by zero.
            xcl_f = t32("xcl_f")
            nc.vector.tensor_copy(xcl_f[:], xcl_i[:])
            ycl_f = t32("ycl_f")
            nc.vector.tensor_copy(ycl_f[:], ycl_i[:])
            wx1 = t32("wx1")
            nc.vector.scalar_tensor_tensor(wx1[:], ix[:], 0.5, xcl_f[:],
                                           add, sub)
            wy1 = t32("wy1")
            nc.vector.scalar_tensor_tensor(wy1[:], iy[:], 0.5, ycl_f[:],
                                           add, sub)
            wx0 = t32("wx0")
            nc.vector.tensor_scalar(wx0[:], wx1[:], -1.0, 1.0, mult, add)
            wy0 = t32("wy0")
            nc.vector.tensor_scalar(wy0[:], wy1[:], -1.0, 1.0, mult, add)

            # validity straight from the continuous coords (parallel to the
            # idx chain): round(i) in [lo, hi] <=> i in [lo-0.5, hi+0.5)
            def valid(f, lo, hi, name):
                c = t32(name + "c")
                nc.vector.tensor_scalar(c[:], f[:], hi + 0.4999, lo - 0.5,
                                        amin, amax)
                v = t32(name)
                nc.vector.tensor_tensor(v[:], c[:], f[:], is_eq)
                return v

            vx0 = valid(ix, XOFF, 27.0 + XOFF, "vx0")
            vx1 = valid(ix, XOFF - 1.0, 26.0 + XOFF, "vx1")
            vy0 = valid(iy, YOFF, 27.0 + YOFF, "vy0")
            vy1 = valid(iy, YOFF - 1.0, 26.0 + YOFF, "vy1")

            def vmul(a, b, name, fp16=False):
                o = wpool.tile(shp, F16 if fp16 else F32, name=name)
                nc.vector.tensor_tensor(o[:], a[:], b[:], mult)
                return o

            wx0v = vmul(wx0, vx0, "wx0v")
            wx1v = vmul(wx1, vx1, "wx1v")
            wy0v = vmul(wy0, vy0, "wy0v")
            wy1v = vmul(wy1, vy1, "wy1v")
            w00 = vmul(wy0v, wx0v, "w00", fp16=True)
            w10 = vmul(wy1v, wx0v, "w10", fp16=True)
            w01 = vmul(wy0v, wx1v, "w01", fp16=True)
            w11 = vmul(wy1v, wx1v, "w11", fp16=True)

            # per-chunk tiles
            ps1 = popool.tile([COUT, NPT], F32, name="ps1")
            ps2 = popool.tile([COUT, NPT], F32, name="ps2")
            out_sb = wpool.tile([COUT, NPT], F32, name="out_sb")

            for ch in range(NCH):
                ga = gas[ch]
                # combine: samp [pt, kk, c] (m = kk*64+c, contiguous writes)
                samp = wpool.tile([128, K2, CIN], F16, name=f"samp{ch}")
                tmp = wpool.tile([128, K2, CIN], F16, name=f"tmp{ch}")
                sv = samp[:]
                tv = tmp[:]

                def bc(wt):
                    return wt[:, ch, :, None].to_broadcast([128, K2, CIN])

                nc.vector.tensor_tensor(sv, ga[:, :, 0:CIN], bc(w00), mult)
                nc.vector.tensor_tensor(tv, ga[:, :, CIN:2 * CIN], bc(w10),
                                        mult)
                nc.vector.tensor_tensor(sv, sv, tv, add)
                nc.vector.tensor_tensor(tv, ga[:, :, 2 * CIN:3 * CIN],
                                        bc(w01), mult)
                nc.vector.tensor_tensor(sv, sv, tv, add)
                nc.vector.tensor_tensor(tv, ga[:, :, 3 * CIN:], bc(w11), mult)
                nc.vector.tensor_tensor(sv, sv, tv, add)

                # PE-transpose samp -> rhs[(kk,c)-blocks, pt]
                rhs = wpool.tile([128, NB, PCH], F16, name=f"rhs{ch}")
                sflat = samp[:].rearrange("p k c -> p (k c)")
                for b in range(NB):
                    mlo, mhi = 128 * b, min(128 * (b + 1), CIN * K2)
                    pstile = ppool.tile([128, 128], F16, tag="tps")
                    nc.tensor.transpose(
                        pstile[:mhi - mlo, :], sflat[:, mlo:mhi], id_sb[:])
                    nc.scalar.copy(
                        rhs[:mhi - mlo, b, :], pstile[:mhi - mlo, :PCH])
                nc.vector.memset(rhs[CIN * K2 - 512:, NB - 1, :], 0.0)

                # final matmuls for this chunk's point columns
                cols = slice(ch * PCH, (ch + 1) * PCH)
                for b in range(NB):
                    nc.tensor.matmul(
                        out=ps1[:, cols], lhsT=wwb_sb[:, b, :],
                        rhs=rhs[:, b, :],
                        start=(b == 0), stop=(b == NB - 1))
                for b in range(NB):
                    nc.tensor.matmul(
                        out=ps2[:, cols], lhsT=wwb_sb[:, NB + b, :],
                        rhs=rhs[:, b, :],
                        start=(b == 0), stop=(b == NB - 1))
                nc.vector.tensor_tensor(out_sb[:, cols], ps1[:, cols],
                                        xcf_sb[:, cols], mult)
                nc.vector.tensor_tensor(out_sb[:, cols], out_sb[:, cols],
                                        ps2[:, cols], add)
                nc.sync.dma_start(out_d.ap()[:, cols], out_sb[:, cols])

    nc.compile()
    return nc


def _host_inputs(x, w_off, b_off, w_wgt, b_wgt):
    """Build the 8 per-core input dicts (layout/shard prep only)."""
    x = np.asarray(x, dtype=np.float32)
    w_off = np.asarray(w_off, dtype=np.float32)
    b_off = np.asarray(b_off, dtype=np.float32)
    w_wgt = np.asarray(w_wgt, dtype=np.float32)
    b_wgt = np.asarray(b_wgt, dtype=np.float32)

    xs = np.linspace(-1.0, 1.0, W).astype(np.float32)
    ys = np.linspace(-1.0, 1.0, H).astype(np.float32)
    kx = np.linspace(-(K - 1) / (W - 1), (K - 1) / (W - 1), K).astype(np.float32)
    ky = np.linspace(-(K - 1) / (H - 1), (K - 1) / (H - 1), K).astype(np.float32)

    # wwb [128, 10, 64] fp16, contraction rows m = kk*64 + c (kk-major, to
    # match the device's contiguous samp layout): chunks 0..4 =
    # W~[m, o] = w_wgt[o, c*9+kk] (zero-pad 576->640), chunks 5..9 =
    # B~[m, o] = b_wgt.reshape(64, 576)[o, c*9+kk].
    m_new = np.arange(K2 * CIN)
    m_old = (m_new % CIN) * K2 + (m_new // CIN)   # (kk,c) -> c*9+kk
    wtp = np.zeros((640, COUT), dtype=np.float32)
    wtp[:576] = w_wgt.T[m_old]
    btp = np.zeros((640, COUT), dtype=np.float32)
    btp[:576] = b_wgt.reshape(CIN, K2 * COUT).T[m_old]
    wwb = np.concatenate([wtp.reshape(5, 128, COUT),
                          btp.reshape(5, 128, COUT)], axis=0)
    wwb = wwb.transpose(1, 0, 2).reshape(128, 10 * COUT).astype(np.float16)

    # idx-wrap permutation selectors: mg[pt, g*128+q] = (pt == g*16 + q%16)
    mgm = np.zeros((128, 8, 128), dtype=np.float16)
    q = np.arange(128)
    for gsel in range(8):
        mgm[gsel * 16 + (q % 16), gsel, q] = 1.0
    mgm = mgm.reshape(128, 8 * 128)

    wofft = np.zeros((128, 2 * K2), dtype=np.float16)
    wofft[:CIN] = w_off.T.astype(np.float16)
    ident = np.eye(128, dtype=np.float16)

    # patch-table row/col clip maps
    rt = np.clip(np.arange(TBL_S) - 1, 0, H - 1)
    rb = np.clip(np.arange(TBL_S), 0, H - 1)
    ct = np.clip(np.arange(TBL_T) - 1, 0, W - 1)
    cr = np.clip(np.arange(TBL_T), 0, W - 1)

    in_maps = []
    for c in range(NCORES):
        n, half = divmod(c, 2)
        r0 = HHALF * half
        xn = x[n]                             # [64, 28, 28]
        x_hwc = xn.transpose(1, 2, 0)         # [28, 28, 64]

        # 2x2 patch table [841, 256] fp16: row (s,t) =
        # [x[rt,ct] | x[rb,ct] | x[rt,cr] | x[rb,cr]]
        tbl = np.concatenate([
            x_hwc[rt][:, ct], x_hwc[rb][:, ct],
            x_hwc[rt][:, cr], x_hwc[rb][:, cr],
        ], axis=-1).astype(np.float16)        # [29, 29, 256]

        xslice = xn.reshape(CIN, H * W)[:, r0 * W:r0 * W + NPT]
        xcpad = np.zeros((128, 512), dtype=np.float16)
        xcpad[:CIN, :NPT] = xslice.astype(np.float16)

        # base grids [128, NCH, K2] with the floor-shift bakes (-0.5 turns
        # the round-to-nearest cast into a floor)
        bx = np.full((128, NCH, K2), SC + XOFF - 0.5, dtype=np.float32)
        by = np.full((128, NCH, K2), SC + YOFF - 0.5, dtype=np.float32)
        p_idx = np.arange(PCH)
        for ch in range(NCH):
            g = r0 * W + ch * PCH + p_idx          # global pixel
            row, col = g // W, g % W
            for kk in range(K2):
                kyi, kxi = divmod(kk, K)
                bx[:PCH, ch, kk] = (xs[col] + kx[kxi] + b_off[2 * kk]
                                    + 1.0) * SC + XOFF - 0.5
                by[:PCH, ch, kk] = (ys[row] + ky[kyi] + b_off[2 * kk + 1]
                                    + 1.0) * SC + YOFF - 0.5

        in_maps.append({
            "tbl": tbl.reshape(TBL_ROWS, 4 * CIN),
            "xc16": xcpad,
            "xcf": np.ascontiguousarray(xslice[:COUT]),
            "wofft": wofft,
            "basex": bx.reshape(128, NCH * K2),
            "basey": by.reshape(128, NCH * K2),
            "wwb": wwb,
            "mg": mgm,
            "ident": ident,
        })
    return in_maps


def get_program():
    if "nc" not in _CACHE:
        _CACHE["nc"] = _build_program()
    return _CACHE["nc"]


def run_cores(in_maps, **kw):
    nc = get_program()
    return run_bass_kernel_spmd(nc, in_maps, core_ids=list(range(NCORES)), **kw)


def assemble(results):
    out = np.zeros((N, COUT, H, W), dtype=np.float32)
    for c in range(NCORES):
        n, half = divmod(c, 2)
        out[n, :, HHALF * half:HHALF * (half + 1), :] = \
            results[c]["out"].reshape(COUT, HHALF, W)
    return out


def kernel(x, w_off, b_off, w_wgt, b_wgt):
    in_maps = _host_inputs(x, w_off, b_off, w_wgt, b_wgt)
    res = run_cores(in_maps)
    return assemble(res.results)


# revision 34
# speedup vs baseline: 1.4955x; 1.0498x over previous
"""Trainium2 Bass kernel for nn_DeformRouting (deformable routing conv).

Sharding: 8 cores, data-parallel over N x H-halves: core c handles image
n = c//2, row-half = c%2 (14 rows x 28 cols = 392 pixels).

Math (per pixel pt, output channel o; m = c*9+kk):
  out[o, pt] = x[o,pt] * sum_m w_wgt[o,m] * samp[m,pt]
             + sum_m b_wgt.reshape(64,576)[o,m] * samp[m,pt]
where samp[(c,kk), pt] is the bilinear sample of x[c] at the deformed
position of tap kk for pixel pt.

Device pipeline per core (points-on-partitions, 4 chunks of 98 pts):
  1. offset conv: 4 PE matmuls -> ps_off[pt, ch, 18] (PSUM, read in place).
  2. index math (short chain, all coords pre-shifted positive so
     floor == i32-trunc): ix = off*13.5 + base' -> floor -> clip ->
     idx = 29*ycl + xcl (fp16, exact ints).
  3. idx wrap to the gather's 16-partition layout: 8 PE perm-matmuls
     (fp16) + one tensor_scalar(-928) PSUM->i16 copy.
  4. per-chunk SWDGE dma_gather (1152 idx x 1KB) from a host-built
     29x29 2x2-PATCH table (fp16): one descriptor fetches all four
     bilinear taps [A0|A1|B0|B1] x 64ch. Chunk gathers pipeline: chunk
     c's DMA + combine run under chunk c+1's descriptor generation.
  5. bilinear weights (fp16, off critical path) + combine (7 DVE ops
     per chunk, fp16).
  6. per-chunk PE transposes (fp16) -> rhs[(c,kk), pt] and 10
     accumulating fp16 matmuls -> ps1/ps2; out = ps1*x + ps2 (f32).
"""

import numpy as np

import concourse.bass as bass
import concourse.tile as tile
from concourse import bacc, mybir
from concourse.bass_utils import run_bass_kernel_spmd

# problem constants (hardcoded per contract)
N, CIN, COUT, H, W, K = 4, 64, 64, 28, 28, 3
K2 = K * K  # 9
NCORES = 8
HHALF = H // 2          # 14 rows per core
NPT = HHALF * W         # 392 points per core
PCH = 98                # points per partition-chunk
NCH = 4                 # chunks (4*98 = 392)
TBL_S = H + 1           # 29 y-floor slots
TBL_T = W + 1           # 29 x-floor slots
TBL_ROWS = TBL_S * TBL_T  # 841 patch rows
SC = (W - 1) / 2.0      # 13.5
XOFF = 30.0             # x floor-shift: xf = round(ix + 30 - 0.5) = floor(ix) + 30
YOFF = 32.0             # y floor-shift: yf = round(iy + 32 - 0.5) = floor(iy) + 32
# The DVE f32->i32 cast rounds to nearest, so the host bakes (XOFF - 0.5)
# into the base grid and the fractional weight adds the 0.5 back.
# idx = 29*(ycl-31) + (xcl-29) = 29*ycl + xcl - 928
IDX_BIAS = 928.0
NI = K2 * 128           # 1152 gather descriptors per chunk
NB = 5                  # ceil(576/128) contraction chunks

F32 = mybir.dt.float32
F16 = mybir.dt.float16
I32 = mybir.dt.int32
I16 = mybir.dt.int16

_CACHE = {}


def _alu(name):
    return getattr(mybir.AluOpType, name)


def _build_program():
    nc = bacc.Bacc("TRN2", target_bir_lowering=False, debug=False,
                   num_devices=NCORES)

    # DRAM I/O (per-core shapes)
    # packed f16 input: [xc16(512) | wofft(18) | ident(128) | wwb(640) |
    #                    mg(1024)] = 2322 f16 per partition
    tbl = nc.dram_tensor("tbl", [TBL_ROWS, 4 * CIN], F16, kind="ExternalInput")
    pf16 = nc.dram_tensor("pf16", [128, 2322], F16, kind="ExternalInput")
    pf32 = nc.dram_tensor("pf32", [128, 2 * NCH * K2], F32,
                          kind="ExternalInput")
    xcf = nc.dram_tensor("xcf", [COUT, NPT], F32, kind="ExternalInput")
    out_d = nc.dram_tensor("out", [COUT, NPT], F32, kind="ExternalOutput")

    mult, add, sub = _alu("mult"), _alu("add"), _alu("subtract")
    is_eq = _alu("is_equal")
    amin, amax = _alu("min"), _alu("max")

    with tile.TileContext(nc) as tc:
        with (
            tc.tile_pool(name="const", bufs=1) as cpool,
            tc.tile_pool(name="work", bufs=1) as wpool,
            tc.tile_pool(name="psoff", bufs=1, space="PSUM") as opool,
            tc.tile_pool(name="pst", bufs=2, space="PSUM") as ppool,
            tc.tile_pool(name="pso", bufs=1, space="PSUM") as popool,
        ):
            # ---- packed input loads (3 DMA dispatches total) ----
            big16 = cpool.tile([128, 2322], F16)
            nc.sync.dma_start(big16[:], pf16.ap())
            big32 = cpool.tile([128, 2, NCH, K2], F32)
            nc.sync.dma_start(big32[:], pf32.ap().rearrange(
                "p (s a b) -> p s a b", s=2, a=NCH))
            xcf_sb = cpool.tile([COUT, NPT], F32)
            nc.sync.dma_start(xcf_sb[:], xcf.ap())

            xc_sb = big16[:, 0:512]
            wofft_v = big16[:, 512:530]
            id_v = big16[:, 530:658]
            wwb_v = big16[:, 658:1298].rearrange("p (a b) -> p a b", a=10)
            mg_v = big16[:, 1298:2322].rearrange("p (a b) -> p a b", a=8)
            basex_v = big32[:, 0]
            basey_v = big32[:, 1]

            # ---- 1. offset conv: ps_off[pt, ch, 18] ----
            ps_off = opool.tile([128, NCH, 2 * K2], F32)
            for ch in range(NCH):
                nc.tensor.matmul(
                    out=ps_off[:, ch, :],
                    lhsT=xc_sb[:, ch * PCH:ch * PCH + 128],
                    rhs=wofft_v,
                    start=True, stop=True,
                )

            # ---- 2. index math (critical path to the gathers) ----
            # The f32->i32 cast rounds to nearest; with the -0.5 host bake
            # that IS the floor. clip commutes with the rounding cast for
            # integer bounds, so clip+floor fuse into one ts-with-cast op.
            shp = [128, NCH, K2]

            def t32(name):
                return wpool.tile(shp, F32, name=name)

            offx = ps_off[:, :, 0:2 * K2:2]
            offy = ps_off[:, :, 1:2 * K2:2]
            ix = t32("ix")
            nc.vector.scalar_tensor_tensor(ix[:], offx, SC, basex_v,
                                           mult, add)
            iy = t32("iy")
            nc.vector.scalar_tensor_tensor(iy[:], offy, SC, basey_v,
                                           mult, add)
            xcl_i = wpool.tile(shp, I32, name="xcl_i")
            nc.vector.tensor_scalar(xcl_i[:], ix[:], 57.0, 29.0, amin, amax)
            ycl_i = wpool.tile(shp, I32, name="ycl_i")
            nc.vector.tensor_scalar(ycl_i[:], iy[:], 59.0, 31.0, amin, amax)
            idxh = wpool.tile(shp, F16, name="idxh")
            nc.vector.scalar_tensor_tensor(idxh[:], ycl_i[:], float(TBL_T),
                                           xcl_i[:], mult, add)

            # ---- 3. wrap idx into the gather's 16-partition layout ----
            psw = opool.tile([128, 8, NCH * K2], F32, name="psw")
            idxv = idxh[:].rearrange("p a b -> p (a b)")
            for gsel in range(8):
                nc.tensor.matmul(
                    out=psw[:, gsel, :], lhsT=mg_v[:, gsel, :], rhs=idxv,
                    start=True, stop=True)
            wrap = wpool.tile([128, NCH * K2, 8], I16, name="wrap")
            for ch in range(NCH):
                nc.vector.tensor_scalar(
                    wrap[:, ch * K2:(ch + 1) * K2, :].rearrange(
                        "q m g -> q g m"),
                    psw[:, :, ch * K2:(ch + 1) * K2], IDX_BIAS, None, sub)

            # ---- 4. per-chunk gathers, emitted before the weight math so
            # their DVE semaphore gate covers only the wrap ops. The last
            # chunk is split kk 0..3 / 4..8 so its combine+transpose tail
            # overlaps the second half's descriptor generation. ----
            KA = 4

            def gather(name, mlo, nk):
                ga = wpool.tile([128, nk, 4 * CIN], F16, name=name)
                nc.gpsimd.dma_gather(
                    out_ap=ga[:],
                    in_ap=tbl.ap(),
                    idxs_ap=wrap[:, mlo:mlo + nk, :].rearrange(
                        "q m g -> q (m g)"),
                    num_idxs=nk * 128, num_idxs_reg=nk * 128,
                    elem_size=4 * CIN, single_packet=False)
                return ga

            gas = [gather(f"ga{ch}", ch * K2, K2) for ch in range(NCH - 1)]
            ga3a = gather("ga3a", (NCH - 1) * K2, KA)
            ga3b = gather("ga3b", (NCH - 1) * K2 + KA, K2 - KA)

            # ---- 4+5. per-chunk gather + combine weights ----
            # bilinear weights (fp16) - consumed only by the combine, so the
            # scheduler runs these during gather descriptor generation.
            # Fractional weights use the CLIPPED floor: wherever the clip
            # bites, both taps of that axis have zero validity, so the
            # wrong magnitude is multiplied by zero.
            xcl_f = t32("xcl_f")
            nc.vector.tensor_copy(xcl_f[:], xcl_i[:])
            ycl_f = t32("ycl_f")
            nc.vector.tensor_copy(ycl_f[:], ycl_i[:])
            wx1 = t32("wx1")
            nc.vector.scalar_tensor_tensor(wx1[:], ix[:], 0.5, xcl_f[:],
                                           add, sub)
            wy1 = t32("wy1")
            nc.vector.scalar_tensor_tensor(wy1[:], iy[:], 0.5, ycl_f[:],
                                           add, sub)
            wx0 = t32("wx0")
            nc.vector.tensor_scalar(wx0[:], wx1[:], -1.0, 1.0, mult, add)
            wy0 = t32("wy0")
            nc.vector.tensor_scalar(wy0[:], wy1[:], -1.0, 1.0, mult, add)

            # validity straight from the continuous coords (parallel to the
            # idx chain): round(i) in [lo, hi] <=> i in [lo-0.5, hi+0.5)
            def valid(f, lo, hi, name):
                c = t32(name + "c")
                nc.vector.tensor_scalar(c[:], f[:], hi + 0.4999, lo - 0.5,
                                        amin, amax)
                v = t32(name)
                nc.vector.tensor_tensor(v[:], c[:], f[:], is_eq)
                return v

            vx0 = valid(ix, XOFF, 27.0 + XOFF, "vx0")
            vx1 = valid(ix, XOFF - 1.0, 26.0 + XOFF, "vx1")
            vy0 = valid(iy, YOFF, 27.0 + YOFF, "vy0")
            vy1 = valid(iy, YOFF - 1.0, 26.0 + YOFF, "vy1")

            def vmul(a, b, name, fp16=False):
                o = wpool.tile(shp, F16 if fp16 else F32, name=name)
                nc.vector.tensor_tensor(o[:], a[:], b[:], mult)
                return o

            wx0v = vmul(wx0, vx0, "wx0v")
            wx1v = vmul(wx1, vx1, "wx1v")
            wy0v = vmul(wy0, vy0, "wy0v")
            wy1v = vmul(wy1, vy1, "wy1v")
            w00 = vmul(wy0v, wx0v, "w00", fp16=True)
            w10 = vmul(wy1v, wx0v, "w10", fp16=True)
            w01 = vmul(wy0v, wx1v, "w01", fp16=True)
            w11 = vmul(wy1v, wx1v, "w11", fp16=True)

            # per-chunk tiles
            ps1 = popool.tile([COUT, NPT], F32, name="ps1")
            ps2 = popool.tile([COUT, NPT], F32, name="ps2")
            out_sb = wpool.tile([COUT, NPT], F32, name="out_sb")

            def combine(samp, tmp, ga, ch, k0, nk):
                # samp view [pt, kk-range, c]; m = kk*64+c contiguous
                sv = samp[:, k0:k0 + nk, :]
                tv = tmp[:, :nk, :]

                def bc(wt):
                    return wt[:, ch, k0:k0 + nk, None].to_broadcast(
                        [128, nk, CIN])

                nc.vector.tensor_tensor(sv, ga[:, :, 0:CIN], bc(w00), mult)
                nc.vector.tensor_tensor(tv, ga[:, :, CIN:2 * CIN], bc(w10),
                                        mult)
                nc.vector.tensor_tensor(sv, sv, tv, add)
                nc.vector.tensor_tensor(tv, ga[:, :, 2 * CIN:3 * CIN],
                                        bc(w01), mult)
                nc.vector.tensor_tensor(sv, sv, tv, add)
                nc.vector.tensor_tensor(tv, ga[:, :, 3 * CIN:], bc(w11), mult)
                nc.vector.tensor_tensor(sv, sv, tv, add)

            def transposes(rhs, samp, blo, bhi):
                sflat = samp[:].rearrange("p k c -> p (k c)")
                for b in range(blo, bhi):
                    mlo, mhi = 128 * b, min(128 * (b + 1), CIN * K2)
                    pstile = ppool.tile([128, 128], F16, tag="tps")
                    nc.tensor.transpose(
                        pstile[:mhi - mlo, :], sflat[:, mlo:mhi], id_v)
                    nc.scalar.copy(
                        rhs[:mhi - mlo, b, :], pstile[:mhi - mlo, :PCH])

            def finals(rhs, ch, blo, bhi):
                cols = slice(ch * PCH, (ch + 1) * PCH)
                for ps, boff in ((ps1, 0), (ps2, NB)):
                    for b in range(blo, bhi):
                        nc.tensor.matmul(
                            out=ps[:, cols], lhsT=wwb_v[:, boff + b, :],
                            rhs=rhs[:, b, :],
                            start=(b == 0), stop=(b == NB - 1))

            def emit_out(ch):
                cols = slice(ch * PCH, (ch + 1) * PCH)
                nc.vector.tensor_tensor(out_sb[:, cols], ps1[:, cols],
                                        xcf_sb[:, cols], mult)
                nc.vector.tensor_tensor(out_sb[:, cols], out_sb[:, cols],
                                        ps2[:, cols], add)
                nc.sync.dma_start(out_d.ap()[:, cols], out_sb[:, cols])

            for ch in range(NCH - 1):
                samp = wpool.tile([128, K2, CIN], F16, name=f"samp{ch}")
                tmp = wpool.tile([128, K2, CIN], F16, name=f"tmp{ch}")
                combine(samp, tmp, gas[ch], ch, 0, K2)
                rhs = wpool.tile([128, NB, PCH], F16, name=f"rhs{ch}")
                transposes(rhs, samp, 0, NB)
                nc.vector.memset(rhs[CIN * K2 - 512:, NB - 1, :], 0.0)
                finals(rhs, ch, 0, NB)
                emit_out(ch)

            # last chunk: halves pipelined (kk 0..3 = m-blocks 0,1;
            # kk 4..8 = m-blocks 2,3,4)
            ch = NCH - 1
            samp = wpool.tile([128, K2, CIN], F16, name=f"samp{ch}")
            tmp = wpool.tile([128, K2, CIN], F16, name=f"tmp{ch}")
            rhs = wpool.tile([128, NB, PCH], F16, name=f"rhs{ch}")
            combine(samp, tmp, ga3a, ch, 0, KA)
            transposes(rhs, samp, 0, 2)
            combine(samp, tmp, ga3b, ch, KA, K2 - KA)
            transposes(rhs, samp, 2, NB)
            nc.vector.memset(rhs[CIN * K2 - 512:, NB - 1, :], 0.0)
            finals(rhs, ch, 0, NB)
            emit_out(ch)

    nc.compile()
    return nc


def _host_inputs(x, w_off, b_off, w_wgt, b_wgt):
    """Build the 8 per-core input dicts (layout/shard prep only)."""
    x = np.asarray(x, dtype=np.float32)
    w_off = np.asarray(w_off, dtype=np.float32)
    b_off = np.asarray(b_off, dtype=np.float32)
    w_wgt = np.asarray(w_wgt, dtype=np.float32)
    b_wgt = np.asarray(b_wgt, dtype=np.float32)

    xs = np.linspace(-1.0, 1.0, W).astype(np.float32)
    ys = np.linspace(-1.0, 1.0, H).astype(np.float32)
    kx = np.linspace(-(K - 1) / (W - 1), (K - 1) / (W - 1), K).astype(np.float32)
    ky = np.linspace(-(K - 1) / (H - 1), (K - 1) / (H - 1), K).astype(np.float32)

    # wwb [128, 10, 64] fp16, contraction rows m = kk*64 + c (kk-major, to
    # match the device's contiguous samp layout): chunks 0..4 =
    # W~[m, o] = w_wgt[o, c*9+kk] (zero-pad 576->640), chunks 5..9 =
    # B~[m, o] = b_wgt.reshape(64, 576)[o, c*9+kk].
    m_new = np.arange(K2 * CIN)
    m_old = (m_new % CIN) * K2 + (m_new // CIN)   # (kk,c) -> c*9+kk
    wtp = np.zeros((640, COUT), dtype=np.float32)
    wtp[:576] = w_wgt.T[m_old]
    btp = np.zeros((640, COUT), dtype=np.float32)
    btp[:576] = b_wgt.reshape(CIN, K2 * COUT).T[m_old]
    wwb = np.concatenate([wtp.reshape(5, 128, COUT),
                          btp.reshape(5, 128, COUT)], axis=0)
    wwb = wwb.transpose(1, 0, 2).reshape(128, 10 * COUT).astype(np.float16)

    # idx-wrap permutation selectors: mg[pt, g*128+q] = (pt == g*16 + q%16)
    mgm = np.zeros((128, 8, 128), dtype=np.float16)
    q = np.arange(128)
    for gsel in range(8):
        mgm[gsel * 16 + (q % 16), gsel, q] = 1.0
    mgm = mgm.reshape(128, 8 * 128)

    wofft = np.zeros((128, 2 * K2), dtype=np.float16)
    wofft[:CIN] = w_off.T.astype(np.float16)
    ident = np.eye(128, dtype=np.float16)

    # patch-table row/col clip maps
    rt = np.clip(np.arange(TBL_S) - 1, 0, H - 1)
    rb = np.clip(np.arange(TBL_S), 0, H - 1)
    ct = np.clip(np.arange(TBL_T) - 1, 0, W - 1)
    cr = np.clip(np.arange(TBL_T), 0, W - 1)

    in_maps = []
    for c in range(NCORES):
        n, half = divmod(c, 2)
        r0 = HHALF * half
        xn = x[n]                             # [64, 28, 28]
        x_hwc = xn.transpose(1, 2, 0)         # [28, 28, 64]

        # 2x2 patch table [841, 256] fp16: row (s,t) =
        # [x[rt,ct] | x[rb,ct] | x[rt,cr] | x[rb,cr]]
        tbl = np.concatenate([
            x_hwc[rt][:, ct], x_hwc[rb][:, ct],
            x_hwc[rt][:, cr], x_hwc[rb][:, cr],
        ], axis=-1).astype(np.float16)        # [29, 29, 256]

        xslice = xn.reshape(CIN, H * W)[:, r0 * W:r0 * W + NPT]
        xcpad = np.zeros((128, 512), dtype=np.float16)
        xcpad[:CIN, :NPT] = xslice.astype(np.float16)

        # base grids [128, NCH, K2] with the floor-shift bakes (-0.5 turns
        # the round-to-nearest cast into a floor)
        bx = np.full((128, NCH, K2), SC + XOFF - 0.5, dtype=np.float32)
        by = np.full((128, NCH, K2), SC + YOFF - 0.5, dtype=np.float32)
        p_idx = np.arange(PCH)
        for ch in range(NCH):
            g = r0 * W + ch * PCH + p_idx          # global pixel
            row, col = g // W, g % W
            for kk in range(K2):
                kyi, kxi = divmod(kk, K)
                bx[:PCH, ch, kk] = (xs[col] + kx[kxi] + b_off[2 * kk]
                                    + 1.0) * SC + XOFF - 0.5
                by[:PCH, ch, kk] = (ys[row] + ky[kyi] + b_off[2 * kk + 1]
                                    + 1.0) * SC + YOFF - 0.5

        pf16 = np.concatenate([xcpad, wofft, ident, wwb, mgm],
                              axis=1)  # [128, 2322]
        pf32 = np.concatenate([bx.reshape(128, NCH * K2),
                               by.reshape(128, NCH * K2)], axis=1)
        in_maps.append({
            "tbl": tbl.reshape(TBL_ROWS, 4 * CIN),
            "pf16": np.ascontiguousarray(pf16),
            "pf32": np.ascontiguousarray(pf32),
            "xcf": np.ascontiguousarray(xslice[:COUT]),
        })
    return in_maps


def get_program():
    if "nc" not in _CACHE:
        _CACHE["nc"] = _build_program()
    return _CACHE["nc"]


def run_cores(in_maps, **kw):
    nc = get_program()
    return run_bass_kernel_spmd(nc, in_maps, core_ids=list(range(NCORES)), **kw)


def assemble(results):
    out = np.zeros((N, COUT, H, W), dtype=np.float32)
    for c in range(NCORES):
        n, half = divmod(c, 2)
        out[n, :, HHALF * half:HHALF * (half + 1), :] = \
            results[c]["out"].reshape(COUT, HHALF, W)
    return out


def kernel(x, w_off, b_off, w_wgt, b_wgt):
    in_maps = _host_inputs(x, w_off, b_off, w_wgt, b_wgt)
    res = run_cores(in_maps)
    return assemble(res.results)


# revision 40
# speedup vs baseline: 1.4969x; 1.0009x over previous
"""Trainium2 Bass kernel for nn_DeformRouting (deformable routing conv).

Sharding: 8 cores, data-parallel over N x H-halves: core c handles image
n = c//2, row-half = c%2 (14 rows x 28 cols = 392 pixels).

Math (per pixel pt, output channel o; m = c*9+kk):
  out[o, pt] = x[o,pt] * sum_m w_wgt[o,m] * samp[m,pt]
             + sum_m b_wgt.reshape(64,576)[o,m] * samp[m,pt]
where samp[(c,kk), pt] is the bilinear sample of x[c] at the deformed
position of tap kk for pixel pt.

Device pipeline per core (points-on-partitions, 4 chunks of 98 pts):
  1. offset conv: 4 PE matmuls -> ps_off[pt, ch, 18] (PSUM, read in place).
  2. index math (short chain, all coords pre-shifted positive so
     floor == i32-trunc): ix = off*13.5 + base' -> floor -> clip ->
     idx = 29*ycl + xcl (fp16, exact ints).
  3. idx wrap to the gather's 16-partition layout: 8 PE perm-matmuls
     (fp16) + one tensor_scalar(-928) PSUM->i16 copy.
  4. per-chunk SWDGE dma_gather (1152 idx x 1KB) from a host-built
     29x29 2x2-PATCH table (fp16): one descriptor fetches all four
     bilinear taps [A0|A1|B0|B1] x 64ch. Chunk gathers pipeline: chunk
     c's DMA + combine run under chunk c+1's descriptor generation.
  5. bilinear weights (fp16, off critical path) + combine (7 DVE ops
     per chunk, fp16).
  6. per-chunk PE transposes (fp16) -> rhs[(c,kk), pt] and 10
     accumulating fp16 matmuls -> ps1/ps2; out = ps1*x + ps2 (f32).
"""

import numpy as np

import concourse.bass as bass
import concourse.tile as tile
from concourse import bacc, mybir
from concourse.bass_utils import run_bass_kernel_spmd

# problem constants (hardcoded per contract)
N, CIN, COUT, H, W, K = 4, 64, 64, 28, 28, 3
K2 = K * K  # 9
NCORES = 8
HHALF = H // 2          # 14 rows per core
NPT = HHALF * W         # 392 points per core
PCH = 98                # points per partition-chunk
NCH = 4                 # chunks (4*98 = 392)
TBL_S = H + 1           # 29 y-floor slots
TBL_T = W + 1           # 29 x-floor slots
TBL_ROWS = TBL_S * TBL_T  # 841 patch rows
SC = (W - 1) / 2.0      # 13.5
XOFF = 30.0             # x floor-shift: xf = round(ix + 30 - 0.5) = floor(ix) + 30
YOFF = 32.0             # y floor-shift: yf = round(iy + 32 - 0.5) = floor(iy) + 32
# The DVE f32->i32 cast rounds to nearest, so the host bakes (XOFF - 0.5)
# into the base grid and the fractional weight adds the 0.5 back.
# idx = 29*(ycl-31) + (xcl-29) = 29*ycl + xcl - 928
IDX_BIAS = 928.0
NI = K2 * 128           # 1152 gather descriptors per chunk
NB = 5                  # ceil(576/128) contraction chunks

F32 = mybir.dt.float32
F16 = mybir.dt.float16
I32 = mybir.dt.int32
I16 = mybir.dt.int16

_CACHE = {}


def _alu(name):
    return getattr(mybir.AluOpType, name)


def _build_program():
    nc = bacc.Bacc("TRN2", target_bir_lowering=False, debug=False,
                   num_devices=NCORES)

    # DRAM I/O (per-core shapes)
    # packed f16 input: [xc16(512) | wofft(18) | ident(128) | wwb(640) |
    #                    mg(1024)] = 2322 f16 per partition
    tbl = nc.dram_tensor("tbl", [TBL_ROWS, 4 * CIN], F16, kind="ExternalInput")
    pf16 = nc.dram_tensor("pf16", [128, 2322], F16, kind="ExternalInput")
    pf32 = nc.dram_tensor("pf32", [128, 2 * NCH * K2], F32,
                          kind="ExternalInput")
    xcf = nc.dram_tensor("xcf", [COUT, NPT], F32, kind="ExternalInput")
    out_d = nc.dram_tensor("out", [COUT, NPT], F32, kind="ExternalOutput")

    mult, add, sub = _alu("mult"), _alu("add"), _alu("subtract")
    is_eq = _alu("is_equal")
    amin, amax = _alu("min"), _alu("max")

    with tile.TileContext(nc) as tc:
        with (
            tc.tile_pool(name="const", bufs=1) as cpool,
            tc.tile_pool(name="work", bufs=1) as wpool,
            tc.tile_pool(name="psoff", bufs=1, space="PSUM") as opool,
            tc.tile_pool(name="pst", bufs=2, space="PSUM") as ppool,
            tc.tile_pool(name="pso", bufs=2, space="PSUM") as popool,
        ):
            # ---- packed input loads (3 DMA dispatches total) ----
            big16 = cpool.tile([128, 2322], F16)
            nc.sync.dma_start(big16[:], pf16.ap())
            big32 = cpool.tile([128, 2, NCH, K2], F32)
            nc.sync.dma_start(big32[:], pf32.ap().rearrange(
                "p (s a b) -> p s a b", s=2, a=NCH))
            xcf_sb = cpool.tile([COUT, NPT], F32)
            nc.sync.dma_start(xcf_sb[:], xcf.ap())

            xc_sb = big16[:, 0:512]
            wofft_v = big16[:, 512:530]
            id_v = big16[:, 530:658]
            wwb_v = big16[:, 658:1298].rearrange("p (a b) -> p a b", a=10)
            mg_v = big16[:, 1298:2322].rearrange("p (a b) -> p a b", a=8)
            basex_v = big32[:, 0]
            basey_v = big32[:, 1]

            # ---- 1. offset conv: ps_off[pt, ch, 18] ----
            ps_off = opool.tile([128, NCH, 2 * K2], F32)
            for ch in range(NCH):
                nc.tensor.matmul(
                    out=ps_off[:, ch, :],
                    lhsT=xc_sb[:, ch * PCH:ch * PCH + 128],
                    rhs=wofft_v,
                    start=True, stop=True,
                )

            # ---- 2. index math (critical path to the gathers) ----
            # The f32->i32 cast rounds to nearest; with the -0.5 host bake
            # that IS the floor. clip commutes with the rounding cast for
            # integer bounds, so clip+floor fuse into one ts-with-cast op.
            shp = [128, NCH, K2]

            def t32(name):
                return wpool.tile(shp, F32, name=name)

            offx = ps_off[:, :, 0:2 * K2:2]
            offy = ps_off[:, :, 1:2 * K2:2]
            ix = t32("ix")
            nc.vector.scalar_tensor_tensor(ix[:], offx, SC, basex_v,
                                           mult, add)
            iy = t32("iy")
            nc.vector.scalar_tensor_tensor(iy[:], offy, SC, basey_v,
                                           mult, add)
            xcl_i = wpool.tile(shp, I32, name="xcl_i")
            nc.vector.tensor_scalar(xcl_i[:], ix[:], 57.0, 29.0, amin, amax)
            ycl_i = wpool.tile(shp, I32, name="ycl_i")
            nc.vector.tensor_scalar(ycl_i[:], iy[:], 59.0, 31.0, amin, amax)
            idxh = wpool.tile(shp, F16, name="idxh")
            nc.vector.scalar_tensor_tensor(idxh[:], ycl_i[:], float(TBL_T),
                                           xcl_i[:], mult, add)

            # ---- 3. wrap idx into the gather's 16-partition layout ----
            psw = opool.tile([128, 8, NCH * K2], F32, name="psw")
            idxv = idxh[:].rearrange("p a b -> p (a b)")
            for gsel in range(8):
                nc.tensor.matmul(
                    out=psw[:, gsel, :], lhsT=mg_v[:, gsel, :], rhs=idxv,
                    start=True, stop=True)
            # one wrap tile per chunk: tile-granularity dependency tracking
            # would otherwise make chunk 0's gather wait for all four
            wraps = []
            for ch in range(NCH):
                w = wpool.tile([128, K2, 8], I16, name=f"wrap{ch}")
                nc.vector.tensor_scalar(
                    w[:].rearrange("q m g -> q g m"),
                    psw[:, :, ch * K2:(ch + 1) * K2], IDX_BIAS, None, sub)
                wraps.append(w)

            # ---- 4. per-chunk gathers, emitted before the weight math so
            # their DVE semaphore gate covers only the wrap ops. The last
            # chunk is split kk 0..3 / 4..8 so its combine+transpose tail
            # overlaps the second half's descriptor generation. ----
            KA = 4

            def gather(name, ch, mlo, nk):
                # trailing pad lanes (partitions 98..127 of the last m
                # column) are trimmed off num_idxs; the untouched SBUF rows
                # become pad columns that the rhs copy drops.
                ga = wpool.tile([128, nk, 4 * CIN], F16, name=name)
                nc.gpsimd.dma_gather(
                    out_ap=ga[:],
                    in_ap=tbl.ap(),
                    idxs_ap=wraps[ch][:, mlo:mlo + nk, :].rearrange(
                        "q m g -> q (m g)"),
                    num_idxs=nk * 128 - 30, num_idxs_reg=nk * 128 - 30,
                    elem_size=4 * CIN, single_packet=False)
                return ga

            gas = [gather(f"ga{ch}", ch, 0, K2) for ch in range(NCH - 1)]
            ga3a = gather("ga3a", NCH - 1, 0, KA)
            ga3b = gather("ga3b", NCH - 1, KA, K2 - KA)

            # ---- 4+5. per-chunk gather + combine weights ----
            # bilinear weights (fp16) - consumed only by the combine, so the
            # scheduler runs these during gather descriptor generation.
            # Fractional weights use the CLIPPED floor: wherever the clip
            # bites, both taps of that axis have zero validity, so the
            # wrong magnitude is multiplied by zero.
            xcl_f = t32("xcl_f")
            nc.vector.tensor_copy(xcl_f[:], xcl_i[:])
            ycl_f = t32("ycl_f")
            nc.vector.tensor_copy(ycl_f[:], ycl_i[:])
            wx1 = t32("wx1")
            nc.vector.scalar_tensor_tensor(wx1[:], ix[:], 0.5, xcl_f[:],
                                           add, sub)
            wy1 = t32("wy1")
            nc.vector.scalar_tensor_tensor(wy1[:], iy[:], 0.5, ycl_f[:],
                                           add, sub)
            wx0 = t32("wx0")
            nc.vector.tensor_scalar(wx0[:], wx1[:], -1.0, 1.0, mult, add)
            wy0 = t32("wy0")
            nc.vector.tensor_scalar(wy0[:], wy1[:], -1.0, 1.0, mult, add)

            # validity straight from the continuous coords (parallel to the
            # idx chain): round(i) in [lo, hi] <=> i in [lo-0.5, hi+0.5)
            def valid(f, lo, hi, name):
                c = t32(name + "c")
                nc.vector.tensor_scalar(c[:], f[:], hi + 0.4999, lo - 0.5,
                                        amin, amax)
                v = t32(name)
                nc.vector.tensor_tensor(v[:], c[:], f[:], is_eq)
                return v

            vx0 = valid(ix, XOFF, 27.0 + XOFF, "vx0")
            vx1 = valid(ix, XOFF - 1.0, 26.0 + XOFF, "vx1")
            vy0 = valid(iy, YOFF, 27.0 + YOFF, "vy0")
            vy1 = valid(iy, YOFF - 1.0, 26.0 + YOFF, "vy1")

            def vmul(a, b, name, fp16=False):
                o = wpool.tile(shp, F16 if fp16 else F32, name=name)
                nc.vector.tensor_tensor(o[:], a[:], b[:], mult)
                return o

            wx0v = vmul(wx0, vx0, "wx0v")
            wx1v = vmul(wx1, vx1, "wx1v")
            wy0v = vmul(wy0, vy0, "wy0v")
            wy1v = vmul(wy1, vy1, "wy1v")
            w00 = vmul(wy0v, wx0v, "w00", fp16=True)
            w10 = vmul(wy1v, wx0v, "w10", fp16=True)
            w01 = vmul(wy0v, wx1v, "w01", fp16=True)
            w11 = vmul(wy1v, wx1v, "w11", fp16=True)


            def combine(samp, ga, ch, k0, nk, tag):
                # tree-structured: 4 independent mults, then 3 adds
                # (depth 3, issues back-to-back on the DVE)
                sv = samp[:]

                def bc(wt):
                    return wt[:, ch, k0:k0 + nk, None].to_broadcast(
                        [128, nk, CIN])

                ts = [wpool.tile([128, nk, CIN], F16, name=f"ct{tag}{j}")
                      for j in range(3)]
                nc.vector.tensor_tensor(sv, ga[:, :, 0:CIN], bc(w00), mult)
                nc.vector.tensor_tensor(ts[0][:], ga[:, :, CIN:2 * CIN],
                                        bc(w10), mult)
                nc.vector.tensor_tensor(ts[1][:], ga[:, :, 2 * CIN:3 * CIN],
                                        bc(w01), mult)
                nc.vector.tensor_tensor(ts[2][:], ga[:, :, 3 * CIN:],
                                        bc(w11), mult)
                nc.vector.tensor_tensor(sv, sv, ts[0][:], add)
                nc.vector.tensor_tensor(ts[1][:], ts[1][:], ts[2][:], add)
                nc.vector.tensor_tensor(sv, sv, ts[1][:], add)

            def transposes(rhs, samp, blo, bhi, b0):
                sflat = samp[:].rearrange("p k c -> p (k c)")
                for b in range(blo, bhi):
                    mlo = 128 * (b - b0)
                    mhi = min(mlo + 128, samp.shape[1] * CIN)
                    pstile = ppool.tile([128, 128], F16, tag="tps")
                    nc.tensor.transpose(
                        pstile[:mhi - mlo, :], sflat[:, mlo:mhi], id_v)
                    nc.scalar.copy(
                        rhs[:mhi - mlo, b, :], pstile[:mhi - mlo, :PCH])

            def finals(rhs, ps1c, ps2c, blo, bhi):
                for ps, boff in ((ps1c, 0), (ps2c, NB)):
                    for b in range(blo, bhi):
                        nc.tensor.matmul(
                            out=ps[:], lhsT=wwb_v[:, boff + b, :],
                            rhs=rhs[:, b, :],
                            start=(b == 0), stop=(b == NB - 1))

            def emit_out(ch, ps1c, ps2c):
                cols = slice(ch * PCH, (ch + 1) * PCH)
                osb = wpool.tile([COUT, PCH], F32, name=f"osb{ch}")
                nc.vector.tensor_tensor(osb[:], ps1c[:], xcf_sb[:, cols],
                                        mult)
                nc.vector.tensor_tensor(osb[:], osb[:], ps2c[:], add)
                nc.sync.dma_start(out_d.ap()[:, cols], osb[:])

            for ch in range(NCH - 1):
                samp = wpool.tile([128, K2, CIN], F16, name=f"samp{ch}")
                combine(samp, gas[ch], ch, 0, K2, f"c{ch}")
                rhs = wpool.tile([128, NB, PCH], F16, name=f"rhs{ch}")
                transposes(rhs, samp, 0, NB, 0)
                nc.vector.memset(rhs[CIN * K2 - 512:, NB - 1, :], 0.0)
                ps1c = popool.tile([COUT, PCH], F32, tag="ps1",
                                   name=f"ps1_{ch}")
                ps2c = popool.tile([COUT, PCH], F32, tag="ps2",
                                   name=f"ps2_{ch}")
                finals(rhs, ps1c, ps2c, 0, NB)
                emit_out(ch, ps1c, ps2c)

            # last chunk: halves pipelined in separate tiles (kk 0..3 =
            # m-blocks 0,1; kk 4..8 = m-blocks 2,3,4)
            ch = NCH - 1
            samp3a = wpool.tile([128, KA, CIN], F16, name="samp3a")
            samp3b = wpool.tile([128, K2 - KA, CIN], F16, name="samp3b")
            rhs = wpool.tile([128, NB, PCH], F16, name=f"rhs{ch}")
            combine(samp3a, ga3a, ch, 0, KA, "c3a")
            transposes(rhs, samp3a, 0, 2, 0)
            combine(samp3b, ga3b, ch, KA, K2 - KA, "c3b")
            transposes(rhs, samp3b, 2, NB, 2)
            nc.vector.memset(rhs[CIN * K2 - 512:, NB - 1, :], 0.0)
            ps1c = popool.tile([COUT, PCH], F32, tag="ps1", name="ps1_3")
            ps2c = popool.tile([COUT, PCH], F32, tag="ps2", name="ps2_3")
            finals(rhs, ps1c, ps2c, 0, NB)
            emit_out(ch, ps1c, ps2c)

    nc.compile()
    return nc


def _host_inputs(x, w_off, b_off, w_wgt, b_wgt):
    """Build the 8 per-core input dicts (layout/shard prep only)."""
    x = np.asarray(x, dtype=np.float32)
    w_off = np.asarray(w_off, dtype=np.float32)
    b_off = np.asarray(b_off, dtype=np.float32)
    w_wgt = np.asarray(w_wgt, dtype=np.float32)
    b_wgt = np.asarray(b_wgt, dtype=np.float32)

    xs = np.linspace(-1.0, 1.0, W).astype(np.float32)
    ys = np.linspace(-1.0, 1.0, H).astype(np.float32)
    kx = np.linspace(-(K - 1) / (W - 1), (K - 1) / (W - 1), K).astype(np.float32)
    ky = np.linspace(-(K - 1) / (H - 1), (K - 1) / (H - 1), K).astype(np.float32)

    # wwb [128, 10, 64] fp16, contraction rows m = kk*64 + c (kk-major, to
    # match the device's contiguous samp layout): chunks 0..4 =
    # W~[m, o] = w_wgt[o, c*9+kk] (zero-pad 576->640), chunks 5..9 =
    # B~[m, o] = b_wgt.reshape(64, 576)[o, c*9+kk].
    m_new = np.arange(K2 * CIN)
    m_old = (m_new % CIN) * K2 + (m_new // CIN)   # (kk,c) -> c*9+kk
    wtp = np.zeros((640, COUT), dtype=np.float32)
    wtp[:576] = w_wgt.T[m_old]
    btp = np.zeros((640, COUT), dtype=np.float32)
    btp[:576] = b_wgt.reshape(CIN, K2 * COUT).T[m_old]
    wwb = np.concatenate([wtp.reshape(5, 128, COUT),
                          btp.reshape(5, 128, COUT)], axis=0)
    wwb = wwb.transpose(1, 0, 2).reshape(128, 10 * COUT).astype(np.float16)

    # idx-wrap permutation selectors: mg[pt, g*128+q] = (pt == g*16 + q%16)
    mgm = np.zeros((128, 8, 128), dtype=np.float16)
    q = np.arange(128)
    for gsel in range(8):
        mgm[gsel * 16 + (q % 16), gsel, q] = 1.0
    mgm = mgm.reshape(128, 8 * 128)

    wofft = np.zeros((128, 2 * K2), dtype=np.float16)
    wofft[:CIN] = w_off.T.astype(np.float16)
    ident = np.eye(128, dtype=np.float16)

    # patch-table row/col clip maps
    rt = np.clip(np.arange(TBL_S) - 1, 0, H - 1)
    rb = np.clip(np.arange(TBL_S), 0, H - 1)
    ct = np.clip(np.arange(TBL_T) - 1, 0, W - 1)
    cr = np.clip(np.arange(TBL_T), 0, W - 1)

    in_maps = []
    for c in range(NCORES):
        n, half = divmod(c, 2)
        r0 = HHALF * half
        xn = x[n]                             # [64, 28, 28]
        x_hwc = xn.transpose(1, 2, 0)         # [28, 28, 64]

        # 2x2 patch table [841, 256] fp16: row (s,t) =
        # [x[rt,ct] | x[rb,ct] | x[rt,cr] | x[rb,cr]]
        tbl = np.concatenate([
            x_hwc[rt][:, ct], x_hwc[rb][:, ct],
            x_hwc[rt][:, cr], x_hwc[rb][:, cr],
        ], axis=-1).astype(np.float16)        # [29, 29, 256]

        xslice = xn.reshape(CIN, H * W)[:, r0 * W:r0 * W + NPT]
        xcpad = np.zeros((128, 512), dtype=np.float16)
        xcpad[:CIN, :NPT] = xslice.astype(np.float16)

        # base grids [128, NCH, K2] with the floor-shift bakes (-0.5 turns
        # the round-to-nearest cast into a floor)
        bx = np.full((128, NCH, K2), SC + XOFF - 0.5, dtype=np.float32)
        by = np.full((128, NCH, K2), SC + YOFF - 0.5, dtype=np.float32)
        p_idx = np.arange(PCH)
        for ch in range(NCH):
            g = r0 * W + ch * PCH + p_idx          # global pixel
            row, col = g // W, g % W
            for kk in range(K2):
                kyi, kxi = divmod(kk, K)
                bx[:PCH, ch, kk] = (xs[col] + kx[kxi] + b_off[2 * kk]
                                    + 1.0) * SC + XOFF - 0.5
                by[:PCH, ch, kk] = (ys[row] + ky[kyi] + b_off[2 * kk + 1]
                                    + 1.0) * SC + YOFF - 0.5

        pf16 = np.concatenate([xcpad, wofft, ident, wwb, mgm],
                              axis=1)  # [128, 2322]
        pf32 = np.concatenate([bx.reshape(128, NCH * K2),
                               by.reshape(128, NCH * K2)], axis=1)
        in_maps.append({
            "tbl": tbl.reshape(TBL_ROWS, 4 * CIN),
            "pf16": np.ascontiguousarray(pf16),
            "pf32": np.ascontiguousarray(pf32),
            "xcf": np.ascontiguousarray(xslice[:COUT]),
        })
    return in_maps


def get_program():
    if "nc" not in _CACHE:
        _CACHE["nc"] = _build_program()
    return _CACHE["nc"]


def run_cores(in_maps, **kw):
    nc = get_program()
    return run_bass_kernel_spmd(nc, in_maps, core_ids=list(range(NCORES)), **kw)


def assemble(results):
    out = np.zeros((N, COUT, H, W), dtype=np.float32)
    for c in range(NCORES):
        n, half = divmod(c, 2)
        out[n, :, HHALF * half:HHALF * (half + 1), :] = \
            results[c]["out"].reshape(COUT, HHALF, W)
    return out


def kernel(x, w_off, b_off, w_wgt, b_wgt):
    in_maps = _host_inputs(x, w_off, b_off, w_wgt, b_wgt)
    res = run_cores(in_maps)
    return assemble(res.results)


# revision 47
# speedup vs baseline: 1.5407x; 1.0293x over previous
"""Trainium2 Bass kernel for nn_DeformRouting (deformable routing conv).

Sharding: 8 cores, data-parallel over N x H-halves: core c handles image
n = c//2, row-half = c%2 (14 rows x 28 cols = 392 pixels).

Math (per pixel pt, output channel o; m = c*9+kk):
  out[o, pt] = x[o,pt] * sum_m w_wgt[o,m] * samp[m,pt]
             + sum_m b_wgt.reshape(64,576)[o,m] * samp[m,pt]
where samp[(c,kk), pt] is the bilinear sample of x[c] at the deformed
position of tap kk for pixel pt.

Device pipeline per core (points-on-partitions, 4 chunks of 98 pts):
  1. offset conv: 4 PE matmuls -> ps_off[pt, ch, 18] (PSUM, read in place).
  2. index math (short chain, all coords pre-shifted positive so
     floor == i32-trunc): ix = off*13.5 + base' -> floor -> clip ->
     idx = 29*ycl + xcl (fp16, exact ints).
  3. idx wrap to the gather's 16-partition layout: 8 PE perm-matmuls
     (fp16) + one tensor_scalar(-928) PSUM->i16 copy.
  4. per-chunk SWDGE dma_gather (1152 idx x 1KB) from a host-built
     29x29 2x2-PATCH table (fp16): one descriptor fetches all four
     bilinear taps [A0|A1|B0|B1] x 64ch. Chunk gathers pipeline: chunk
     c's DMA + combine run under chunk c+1's descriptor generation.
  5. bilinear weights (fp16, off critical path) + combine (7 DVE ops
     per chunk, fp16).
  6. per-chunk PE transposes (fp16) -> rhs[(c,kk), pt] and 10
     accumulating fp16 matmuls -> ps1/ps2; out = ps1*x + ps2 (f32).
"""

import numpy as np

import concourse.bass as bass
import concourse.tile as tile
from concourse import bacc, mybir
from concourse.bass_utils import run_bass_kernel_spmd

# problem constants (hardcoded per contract)
N, CIN, COUT, H, W, K = 4, 64, 64, 28, 28, 3
K2 = K * K  # 9
NCORES = 8
HHALF = H // 2          # 14 rows per core
NPT = HHALF * W         # 392 points per core
PCH = 98                # points per partition-chunk
NCH = 4                 # chunks (4*98 = 392)
TBL_S = H + 1           # 29 y-floor slots
TBL_T = W + 1           # 29 x-floor slots
TBL_ROWS = TBL_S * TBL_T  # 841 patch rows
SC = (W - 1) / 2.0      # 13.5
XOFF = 30.0             # x floor-shift: xf = round(ix + 30 - 0.5) = floor(ix) + 30
YOFF = 32.0             # y floor-shift: yf = round(iy + 32 - 0.5) = floor(iy) + 32
# The DVE f32->i32 cast rounds to nearest, so the host bakes (XOFF - 0.5)
# into the base grid and the fractional weight adds the 0.5 back.
# idx = 29*(ycl-31) + (xcl-29) = 29*ycl + xcl - 928
IDX_BIAS = 928.0
NI = K2 * 128           # 1152 gather descriptors per chunk
NB = 5                  # ceil(576/128) contraction chunks

F32 = mybir.dt.float32
F16 = mybir.dt.float16
I32 = mybir.dt.int32
I16 = mybir.dt.int16

_CACHE = {}


def _alu(name):
    return getattr(mybir.AluOpType, name)


def _build_program():
    nc = bacc.Bacc("TRN2", target_bir_lowering=False, debug=False,
                   num_devices=NCORES)

    # DRAM I/O (per-core shapes)
    # packed f16 input: [xc16(512) | wofft(18) | ident(128) | wwb(640) |
    #                    mg(1024)] = 2322 f16 per partition
    tbl = nc.dram_tensor("tbl", [TBL_ROWS, 4 * CIN], F16, kind="ExternalInput")
    pf16 = nc.dram_tensor("pf16", [128, 2322], F16, kind="ExternalInput")
    pf32 = nc.dram_tensor("pf32", [128, 2 * NCH * K2], F32,
                          kind="ExternalInput")
    xcf = nc.dram_tensor("xcf", [COUT, NPT], F32, kind="ExternalInput")
    out_d = nc.dram_tensor("out", [COUT, NPT], F32, kind="ExternalOutput")

    mult, add, sub = _alu("mult"), _alu("add"), _alu("subtract")
    is_eq = _alu("is_equal")
    amin, amax = _alu("min"), _alu("max")

    with tile.TileContext(nc) as tc:
        with (
            tc.tile_pool(name="const", bufs=1) as cpool,
            tc.tile_pool(name="work", bufs=1) as wpool,
            tc.tile_pool(name="psoff", bufs=1, space="PSUM") as opool,
            tc.tile_pool(name="pst", bufs=2, space="PSUM") as ppool,
            tc.tile_pool(name="pso", bufs=2, space="PSUM") as popool,
        ):
            # ---- packed input loads (3 DMA dispatches total) ----
            big16 = cpool.tile([128, 2322], F16)
            nc.sync.dma_start(big16[:], pf16.ap())
            big32 = cpool.tile([128, NCH, 2 * K2], F32)
            nc.sync.dma_start(big32[:], pf32.ap().rearrange(
                "p (a b) -> p a b", a=NCH))
            xcf_sb = cpool.tile([COUT, NPT], F32)
            nc.sync.dma_start(xcf_sb[:], xcf.ap())

            xc_sb = big16[:, 0:512]
            wofft_v = big16[:, 512:530]
            id_v = big16[:, 530:658]
            wwb_v = big16[:, 658:1298].rearrange("p (a b) -> p a b", a=10)
            mg_v = big16[:, 1298:2322].rearrange("p (a b) -> p a b", a=8)
            baseb_v = big32[:]

            # ---- 1. offset conv: ps_off[pt, ch, 18] ----
            ps_off = opool.tile([128, NCH, 2 * K2], F32)
            for ch in range(NCH):
                nc.tensor.matmul(
                    out=ps_off[:, ch, :],
                    lhsT=xc_sb[:, ch * PCH:ch * PCH + 128],
                    rhs=wofft_v,
                    start=True, stop=True,
                )

            # ---- 2. index math (critical path to the gathers) ----
            # The f32->i32 cast rounds to nearest; with the -0.5 host bake
            # that IS the floor. clip commutes with the rounding cast for
            # integer bounds, so clip+floor fuse into one ts-with-cast op.
            # x and y stay interleaved ([128, NCH, 18], x even / y odd) so
            # one op covers both axes wherever the scalars allow.
            shp2 = [128, NCH, 2 * K2]
            ib = wpool.tile(shp2, F32, name="ib")
            nc.vector.scalar_tensor_tensor(ib[:], ps_off[:], SC, baseb_v,
                                           mult, add)
            cl_i = wpool.tile(shp2, I32, name="cl_i")
            nc.vector.tensor_scalar(cl_i[:, :, 0::2], ib[:, :, 0::2],
                                    57.0, 29.0, amin, amax)
            nc.vector.tensor_scalar(cl_i[:, :, 1::2], ib[:, :, 1::2],
                                    59.0, 31.0, amin, amax)
            idxh = wpool.tile([128, NCH, K2], F16, name="idxh")
            nc.vector.scalar_tensor_tensor(idxh[:], cl_i[:, :, 1::2],
                                           float(TBL_T), cl_i[:, :, 0::2],
                                           mult, add)

            # ---- 3. wrap idx into the gather's 16-partition layout ----
            psw = opool.tile([128, 8, NCH * K2], F32, name="psw")
            idxv = idxh[:].rearrange("p a b -> p (a b)")
            for gsel in range(8):
                nc.tensor.matmul(
                    out=psw[:, gsel, :], lhsT=mg_v[:, gsel, :], rhs=idxv,
                    start=True, stop=True)
            # one wrap tile per chunk: tile-granularity dependency tracking
            # would otherwise make chunk 0's gather wait for all four
            wraps = []
            for ch in range(NCH):
                w = wpool.tile([128, K2, 8], I16, name=f"wrap{ch}")
                nc.vector.tensor_scalar(
                    w[:].rearrange("q m g -> q g m"),
                    psw[:, :, ch * K2:(ch + 1) * K2], IDX_BIAS, None, sub)
                wraps.append(w)

            # ---- 4. per-chunk gathers, emitted before the weight math so
            # their DVE semaphore gate covers only the wrap ops. The last
            # chunk is split kk 0..3 / 4..8 so its combine+transpose tail
            # overlaps the second half's descriptor generation. ----
            KA = 4

            def gather(name, ch, mlo, nk):
                # trailing pad lanes (partitions 98..127 of the last m
                # column) are trimmed off num_idxs; the untouched SBUF rows
                # become pad columns that the rhs copy drops.
                ga = wpool.tile([128, nk, 4 * CIN], F16, name=name)
                nc.gpsimd.dma_gather(
                    out_ap=ga[:],
                    in_ap=tbl.ap(),
                    idxs_ap=wraps[ch][:, mlo:mlo + nk, :].rearrange(
                        "q m g -> q (m g)"),
                    num_idxs=nk * 128, num_idxs_reg=nk * 128,
                    elem_size=4 * CIN, single_packet=False)
                return ga

            gas = [gather(f"ga{ch}", ch, 0, K2) for ch in range(NCH - 1)]
            ga3a = gather("ga3a", NCH - 1, 0, KA)
            ga3b = gather("ga3b", NCH - 1, KA, K2 - KA)

            # ---- 5. bilinear weights (fp16), consumed only by the combine.
            # No validity ops: out-of-range taps hit zero-sentinel halves of
            # the patch table (t/s == 0 or 28), and the frac weights are
            # clamped to [0,1] so far-out-of-range weights collapse to 0.
            cl_f = wpool.tile(shp2, F32, name="cl_f")
            nc.vector.tensor_copy(cl_f[:], cl_i[:])
            w1b = wpool.tile(shp2, F32, name="w1b")
            nc.vector.scalar_tensor_tensor(w1b[:], ib[:], 0.5, cl_f[:],
                                           add, sub)
            nc.vector.tensor_scalar(w1b[:], w1b[:], 1.0, 0.0, amin, amax)
            w0b = wpool.tile(shp2, F32, name="w0b")
            nc.vector.scalar_tensor_tensor(w0b[:], cl_f[:], 0.5, ib[:],
                                           add, sub)
            nc.vector.tensor_scalar(w0b[:], w0b[:], 1.0, 0.0, amin, amax)

            def tap(wy, wx, name):
                o = wpool.tile([128, NCH, K2], F16, name=name)
                nc.vector.tensor_tensor(o[:], wy[:, :, 1::2], wx[:, :, 0::2],
                                        mult)
                return o

            w00 = tap(w0b, w0b, "w00")
            w10 = tap(w1b, w0b, "w10")
            w01 = tap(w0b, w1b, "w01")
            w11 = tap(w1b, w1b, "w11")


            def combine(samp, ga, ch, k0, nk, tag):
                # tree-structured: 4 independent mults, then 3 adds
                # (depth 3, issues back-to-back on the DVE)
                sv = samp[:]

                def bc(wt):
                    return wt[:, ch, k0:k0 + nk, None].to_broadcast(
                        [128, nk, CIN])

                ts = [wpool.tile([128, nk, CIN], F16, name=f"ct{tag}{j}")
                      for j in range(3)]
                nc.vector.tensor_tensor(sv, ga[:, :, 0:CIN], bc(w00), mult)
                nc.vector.tensor_tensor(ts[0][:], ga[:, :, CIN:2 * CIN],
                                        bc(w10), mult)
                nc.vector.tensor_tensor(ts[1][:], ga[:, :, 2 * CIN:3 * CIN],
                                        bc(w01), mult)
                nc.vector.tensor_tensor(ts[2][:], ga[:, :, 3 * CIN:],
                                        bc(w11), mult)
                nc.vector.tensor_tensor(sv, sv, ts[0][:], add)
                nc.vector.tensor_tensor(ts[1][:], ts[1][:], ts[2][:], add)
                nc.vector.tensor_tensor(sv, sv, ts[1][:], add)

            def transposes(rhs, samp, blo, bhi, b0):
                sflat = samp[:].rearrange("p k c -> p (k c)")
                for b in range(blo, bhi):
                    mlo = 128 * (b - b0)
                    mhi = min(mlo + 128, samp.shape[1] * CIN)
                    pstile = ppool.tile([128, 128], F16, tag="tps")
                    nc.tensor.transpose(
                        pstile[:mhi - mlo, :], sflat[:, mlo:mhi], id_v)
                    nc.scalar.copy(
                        rhs[:mhi - mlo, b, :], pstile[:mhi - mlo, :PCH])

            def finals(rhs, ps1c, ps2c, blo, bhi):
                for ps, boff in ((ps1c, 0), (ps2c, NB)):
                    for b in range(blo, bhi):
                        nc.tensor.matmul(
                            out=ps[:], lhsT=wwb_v[:, boff + b, :],
                            rhs=rhs[:, b, :],
                            start=(b == 0), stop=(b == NB - 1))

            def emit_out(ch, ps1c, ps2c):
                cols = slice(ch * PCH, (ch + 1) * PCH)
                osb = wpool.tile([COUT, PCH], F32, name=f"osb{ch}")
                nc.vector.tensor_tensor(osb[:], ps1c[:], xcf_sb[:, cols],
                                        mult)
                nc.vector.tensor_tensor(osb[:], osb[:], ps2c[:], add)
                nc.sync.dma_start(out_d.ap()[:, cols], osb[:])

            for ch in range(NCH - 1):
                samp = wpool.tile([128, K2, CIN], F16, name=f"samp{ch}")
                combine(samp, gas[ch], ch, 0, K2, f"c{ch}")
                rhs = wpool.tile([128, NB, PCH], F16, name=f"rhs{ch}")
                transposes(rhs, samp, 0, NB, 0)
                nc.vector.memset(rhs[CIN * K2 - 512:, NB - 1, :], 0.0)
                ps1c = popool.tile([COUT, PCH], F32, tag="ps1",
                                   name=f"ps1_{ch}")
                ps2c = popool.tile([COUT, PCH], F32, tag="ps2",
                                   name=f"ps2_{ch}")
                finals(rhs, ps1c, ps2c, 0, NB)
                emit_out(ch, ps1c, ps2c)

            # last chunk: halves pipelined in separate tiles (kk 0..3 =
            # m-blocks 0,1; kk 4..8 = m-blocks 2,3,4)
            ch = NCH - 1
            samp3a = wpool.tile([128, KA, CIN], F16, name="samp3a")
            samp3b = wpool.tile([128, K2 - KA, CIN], F16, name="samp3b")
            rhs = wpool.tile([128, NB, PCH], F16, name=f"rhs{ch}")
            combine(samp3a, ga3a, ch, 0, KA, "c3a")
            transposes(rhs, samp3a, 0, 2, 0)
            combine(samp3b, ga3b, ch, KA, K2 - KA, "c3b")
            transposes(rhs, samp3b, 2, NB, 2)
            nc.vector.memset(rhs[CIN * K2 - 512:, NB - 1, :], 0.0)
            ps1c = popool.tile([COUT, PCH], F32, tag="ps1", name="ps1_3")
            ps2c = popool.tile([COUT, PCH], F32, tag="ps2", name="ps2_3")
            finals(rhs, ps1c, ps2c, 0, NB)
            emit_out(ch, ps1c, ps2c)

    nc.compile()
    return nc


def _host_inputs(x, w_off, b_off, w_wgt, b_wgt):
    """Build the 8 per-core input dicts (layout/shard prep only)."""
    x = np.asarray(x, dtype=np.float32)
    w_off = np.asarray(w_off, dtype=np.float32)
    b_off = np.asarray(b_off, dtype=np.float32)
    w_wgt = np.asarray(w_wgt, dtype=np.float32)
    b_wgt = np.asarray(b_wgt, dtype=np.float32)

    xs = np.linspace(-1.0, 1.0, W).astype(np.float32)
    ys = np.linspace(-1.0, 1.0, H).astype(np.float32)
    kx = np.linspace(-(K - 1) / (W - 1), (K - 1) / (W - 1), K).astype(np.float32)
    ky = np.linspace(-(K - 1) / (H - 1), (K - 1) / (H - 1), K).astype(np.float32)

    # wwb [128, 10, 64] fp16, contraction rows m = kk*64 + c (kk-major, to
    # match the device's contiguous samp layout): chunks 0..4 =
    # W~[m, o] = w_wgt[o, c*9+kk] (zero-pad 576->640), chunks 5..9 =
    # B~[m, o] = b_wgt.reshape(64, 576)[o, c*9+kk].
    m_new = np.arange(K2 * CIN)
    m_old = (m_new % CIN) * K2 + (m_new // CIN)   # (kk,c) -> c*9+kk
    wtp = np.zeros((640, COUT), dtype=np.float32)
    wtp[:576] = w_wgt.T[m_old]
    btp = np.zeros((640, COUT), dtype=np.float32)
    btp[:576] = b_wgt.reshape(CIN, K2 * COUT).T[m_old]
    wwb = np.concatenate([wtp.reshape(5, 128, COUT),
                          btp.reshape(5, 128, COUT)], axis=0)
    wwb = wwb.transpose(1, 0, 2).reshape(128, 10 * COUT).astype(np.float16)

    # idx-wrap permutation selectors: mg[pt, g*128+q] = (pt == g*16 + q%16)
    mgm = np.zeros((128, 8, 128), dtype=np.float16)
    q = np.arange(128)
    for gsel in range(8):
        mgm[gsel * 16 + (q % 16), gsel, q] = 1.0
    mgm = mgm.reshape(128, 8 * 128)

    wofft = np.zeros((128, 2 * K2), dtype=np.float16)
    wofft[:CIN] = w_off.T.astype(np.float16)
    ident = np.eye(128, dtype=np.float16)

    # patch-table row/col clip maps
    rt = np.clip(np.arange(TBL_S) - 1, 0, H - 1)
    rb = np.clip(np.arange(TBL_S), 0, H - 1)
    ct = np.clip(np.arange(TBL_T) - 1, 0, W - 1)
    cr = np.clip(np.arange(TBL_T), 0, W - 1)

    in_maps = []
    for c in range(NCORES):
        n, half = divmod(c, 2)
        r0 = HHALF * half
        xn = x[n]                             # [64, 28, 28]
        x_hwc = xn.transpose(1, 2, 0)         # [28, 28, 64]

        # 2x2 patch table [841, 256] fp16: row (s,t) =
        # [x[rt,ct] | x[rb,ct] | x[rt,cr] | x[rb,cr]] with zero sentinels
        # where a tap is out of range (replaces on-device validity math)
        tbl = np.concatenate([
            x_hwc[rt][:, ct], x_hwc[rb][:, ct],
            x_hwc[rt][:, cr], x_hwc[rb][:, cr],
        ], axis=-1).astype(np.float16)        # [29, 29, 256]
        tbl[:, 0, 0:128] = 0       # t=0: x0 = -1 -> A0, A1 zero
        tbl[:, TBL_T - 1, 128:256] = 0  # t=28: x1 = 28 -> B0, B1 zero
        tbl[0, :, 0:64] = 0        # s=0: y0 = -1 -> A0 zero
        tbl[0, :, 128:192] = 0     # s=0: B0 zero
        tbl[TBL_S - 1, :, 64:128] = 0   # s=28: y1 = 28 -> A1 zero
        tbl[TBL_S - 1, :, 192:256] = 0  # s=28: B1 zero

        xslice = xn.reshape(CIN, H * W)[:, r0 * W:r0 * W + NPT]
        xcpad = np.zeros((128, 512), dtype=np.float16)
        xcpad[:CIN, :NPT] = xslice.astype(np.float16)

        # base grids [128, NCH, K2] with the floor-shift bakes (-0.5 turns
        # the round-to-nearest cast into a floor)
        bx = np.full((128, NCH, K2), SC + XOFF - 0.5, dtype=np.float32)
        by = np.full((128, NCH, K2), SC + YOFF - 0.5, dtype=np.float32)
        p_idx = np.arange(PCH)
        for ch in range(NCH):
            g = r0 * W + ch * PCH + p_idx          # global pixel
            row, col = g // W, g % W
            for kk in range(K2):
                kyi, kxi = divmod(kk, K)
                bx[:PCH, ch, kk] = (xs[col] + kx[kxi] + b_off[2 * kk]
                                    + 1.0) * SC + XOFF - 0.5
                by[:PCH, ch, kk] = (ys[row] + ky[kyi] + b_off[2 * kk + 1]
                                    + 1.0) * SC + YOFF - 0.5

        pf16 = np.concatenate([xcpad, wofft, ident, wwb, mgm],
                              axis=1)  # [128, 2322]
        # interleave x/y bases: [128, NCH, 18] with x at even, y at odd
        bb = np.empty((128, NCH, 2 * K2), dtype=np.float32)
        bb[:, :, 0::2] = bx
        bb[:, :, 1::2] = by
        pf32 = bb.reshape(128, 2 * NCH * K2)
        in_maps.append({
            "tbl": tbl.reshape(TBL_ROWS, 4 * CIN),
            "pf16": np.ascontiguousarray(pf16),
            "pf32": np.ascontiguousarray(pf32),
            "xcf": np.ascontiguousarray(xslice[:COUT]),
        })
    return in_maps


def get_program():
    if "nc" not in _CACHE:
        _CACHE["nc"] = _build_program()
    return _CACHE["nc"]


def run_cores(in_maps, **kw):
    nc = get_program()
    return run_bass_kernel_spmd(nc, in_maps, core_ids=list(range(NCORES)), **kw)


def assemble(results):
    out = np.zeros((N, COUT, H, W), dtype=np.float32)
    for c in range(NCORES):
        n, half = divmod(c, 2)
        out[n, :, HHALF * half:HHALF * (half + 1), :] = \
            results[c]["out"].reshape(COUT, HHALF, W)
    return out


def kernel(x, w_off, b_off, w_wgt, b_wgt):
    in_maps = _host_inputs(x, w_off, b_off, w_wgt, b_wgt)
    res = run_cores(in_maps)
    return assemble(res.results)
